# revision 1
# baseline (speedup 1.0000x reference)
"""2-layer GAT + MLP head on 8 TRN2 NeuronCores.

Strategy (matches the dst-sharding hint):
- Nodes padded to NP=20480; each core owns a contiguous 2560-dst shard.
- Edges (incl. self-loops, PyG mean-fill edge attr) are sorted by dst,
  grouped into 128-dst tiles, padded per tile-slot to a chunk count K_t
  shared by all cores (SPMD: one program).
- Per layer a node table [NP, 640] bf16 lives in HBM:
  cols [0:512) = h (bf16), bytes [1024:1040) = alpha_src (f32 bits),
  bytes [1040:1056) = alpha_dst (f32 bits). Table rows are permuted so
  the layer-1 table can be assembled by NAG group-wise AllGathers that
  overlap the layer-0 aggregation.
- Aggregation: dma_gather rows by src (h+alphas) and the 256B alpha
  sub-row by dst; per 128-edge chunk compute p = exp(leakyrelu(
  asrc+adst+aedge)) (batched per super-chunk) and accumulate out[dst]
  via a one-hot matmul in PSUM; softmax denominator via a second matmul
  with rhs=p.
- alpha_edge = ew * k[h] is host-folded; alpha_src/dst projections are
  host-folded into the layer weight matrices as extra columns.
- Layer-1 linear runs per dst tile right after its layer-0 finalize;
  each NT/NAG tile group is AllGathered as soon as it is ready.
"""

import numpy as np
import ml_dtypes

import concourse.bacc as bacc
import concourse.bass as bass
import concourse.mybir as mybir
import concourse.tile as tile
from concourse.bass_utils import run_bass_kernel_spmd

F32 = mybir.dt.float32
BF16 = mybir.dt.bfloat16
I16 = mybir.dt.int16
AF = mybir.ActivationFunctionType
OP = mybir.AluOpType

NCORES = 8
SCC = 8  # chunks (of 128 edges) per gather super-chunk


def _bcast4(ap_tile, j, reps):
    """[128, SCC, 4] tile -> [128, 4, reps] zero-step broadcast AP of slot j."""
    sl = ap_tile[:, j, :]
    return bass.AP(sl.tensor, sl.offset, [list(sl.ap[0]), list(sl.ap[-1]), [0, reps]])


def _build_program(NP, F_IN, HC, H, C, NT, K_t, FTS, NAG, use_b0, use_b1, use_l0b, use_l1b):
    NCHUNK = int(sum(K_t))
    E_pad = NCHUNK * 128
    SW = E_pad // 16
    TW = HC + 128  # bf16 table row: h | asrc,adst (f32 bits) | pad
    KB = HC // 128
    GPG = NT // NAG          # dst tiles per allgather group
    GR = GPG * 128           # rows per group per core

    nc = bacc.Bacc(dynamic_dma_scratch_size=65536, num_swdge_queues=4)
    P = nc.declare_dram_parameter

    xT = P("xT", [F_IN, NP], BF16, isOutput=False)
    r0h = P("r0h", [F_IN, HC], BF16, isOutput=False)
    r0a = P("r0a", [F_IN, 8], BF16, isOutput=False)
    r1h = P("r1h", [HC, HC], BF16, isOutput=False)
    r1a = P("r1a", [HC, 8], BF16, isOutput=False)
    r2 = P("r2", [HC, FTS], F32, isOutput=False)
    r3 = P("r3", [FTS, 1], F32, isOutput=False)
    b0t = P("b0t", [128, HC], F32, isOutput=False)
    b1t = P("b1t", [128, HC], F32, isOutput=False)
    l0bt = P("l0bt", [128, FTS], F32, isOutput=False)
    l1bt = P("l1bt", [128, 1], F32, isOutput=False)
    ident = P("ident", [128, 128], F32, isOutput=False)
    srcw = P("srcw", [128, SW], I16, isOutput=False)
    trw = P("trw", [128, NT * 8], I16, isOutput=False)
    ohb = P("ohb", [128, NCHUNK * 128], BF16, isOutput=False)
    oht = P("oht", [128, NCHUNK * 128], F32, isOutput=False)
    ae0 = P("ae0", [128, NCHUNK, 4], F32, isOutput=False)
    ae1 = P("ae1", [128, NCHUNK, 4], F32, isOutput=False)
    outp = P("out", [NT * 128, 1], F32, isOutput=True)

    with tile.TileContext(nc) as tc:
        with (
            tc.tile_pool(name="const", bufs=1) as const,
            tc.tile_pool(name="stage", bufs=2) as stage,
            tc.tile_pool(name="work", bufs=3) as work,
            tc.tile_pool(name="tp", bufs=6) as tp,
            tc.tile_pool(name="adp", bufs=1) as adp,
            tc.tile_pool(name="psacc", bufs=3, space="PSUM") as psacc,
            tc.tile_pool(name="pss", bufs=2, space="PSUM") as pss,
            tc.tile_pool(name="pstr", bufs=3, space="PSUM") as pstr,
            tc.tile_pool(name="dram", bufs=1, space="DRAM") as dram,
        ):
            H0 = dram.tile([NP, TW], BF16, tag="H0")
            H1 = dram.tile([NP, TW], BF16, tag="H1")
            H1g = [dram.tile([GR, TW], BF16, tag=f"H1g{g}", name=f"H1g{g}")
                   for g in range(NAG)]

            _cn = [0]

            def cload(ap_in, shape, dt=F32, tag=None):
                _cn[0] += 1
                cname = tag or f"c{_cn[0]}"
                t = const.tile(shape, dt, tag=cname, name=f"{cname}_{_cn[0]}")
                nc.sync.dma_start(out=t[:], in_=ap_in)
                return t

            r0h_s = cload(r0h[:, :], [F_IN, HC], BF16)
            r0a_s = cload(r0a[:, :], [F_IN, 8], BF16)
            r1h_s = [cload(r1h[k * 128:(k + 1) * 128, :], [128, HC], BF16)
                     for k in range(KB)]
            r1a_s = [cload(r1a[k * 128:(k + 1) * 128, :], [128, 8], BF16)
                     for k in range(KB)]
            r2_s = [cload(r2[k * 128:(k + 1) * 128, :], [128, FTS]) for k in range(KB)]
            r3_s = cload(r3[:, :], [FTS, 1])
            b0_s = cload(b0t[:, :], [128, HC])
            b1_s = cload(b1t[:, :], [128, HC])
            l0b_s = cload(l0bt[:, :], [128, FTS])
            l1b_s = cload(l1bt[:, :], [128, 1])
            id_s = cload(ident[:, :], [128, 128])
            srcw_s = cload(srcw[:, :], [128, SW], I16)
            trw_s = cload(trw[:, :], [128, NT * 8], I16)
            ae0_s = cload(ae0[:, :, :], [128, NCHUNK, 4], tag="ae")

            # ---- phase A: layer-0 table (xT comes pre-permuted to row order)
            MT = NP // 128
            nc._state.push_named_scope("phaseA")
            for mr in range(MT):
                lx = stage.tile([F_IN, 128], BF16, tag="lx")
                nc.sync.dma_start(out=lx[:], in_=xT[:, mr * 128:(mr + 1) * 128])
                ph = psacc.tile([128, HC], F32, tag="ph")
                nc.tensor.matmul(ph[:], lx[:], r0h_s[:], start=True, stop=True)
                pa = pss.tile([128, 8], F32, tag="pss")
                nc.tensor.matmul(pa[:], lx[:], r0a_s[:], start=True, stop=True)
                st = stage.tile([128, TW], BF16, tag="hrow")
                if mr % 2 == 0:
                    nc.vector.tensor_copy(st[:, 0:HC], ph[:])
                else:
                    nc.scalar.activation(st[:, 0:HC], ph[:], AF.Copy)
                nc.vector.tensor_copy(st[:, HC:HC + 16].bitcast(F32), pa[:])
                nc.sync.dma_start(out=H0[mr * 128:(mr + 1) * 128, :], in_=st[:])
            nc._state.pop_named_scope("phaseA")

            # ---- aggregation over one layer's edges ----
            def agg_layer(tbl, ae_s, finalize):
                # chunk q -> owning dst tile
                t_of_q = []
                for t in range(NT):
                    t_of_q += [t] * K_t[t]
                adts = {}

                def ensure_super(s):
                    cnt = min(SCC * 128, E_pad - s * SCC * 128)
                    nch = cnt // 128
                    gA = stage.tile([128, SCC, TW], BF16, tag="gA", name="gA", bufs=3)
                    c0 = s * SCC * 8
                    qparts = []
                    base = 0
                    nq = min(4, nch)
                    for qi in range(nq):
                        take = (nch - base + (nq - qi) - 1) // (nq - qi)
                        qparts.append((base, take, qi))
                        base += take
                    for (b0, tk, qi) in qparts:
                        nc.gpsimd.dma_gather(
                            gA[:, b0:b0 + tk, :], tbl[:, :],
                            srcw_s[:, c0 + b0 * 8:c0 + (b0 + tk) * 8],
                            tk * 128, tk * 128, TW,
                            single_packet=False, queue_num=qi)
                    ohb_t = stage.tile([128, SCC * 128], BF16, tag="ohb", name="ohb_t", bufs=3)
                    nc.sync.dma_start(
                        out=ohb_t[:, 0:nch * 128],
                        in_=ohb[:, s * SCC * 128:s * SCC * 128 + nch * 128])
                    oht_t = stage.tile([128, SCC * 128], F32, tag="oht", name="oht_t", bufs=1)
                    nc.sync.dma_start(
                        out=oht_t[:, 0:nch * 128],
                        in_=oht[:, s * SCC * 128:s * SCC * 128 + nch * 128])
                    # expand alpha_dst per chunk via ohT matmul
                    pead = pstr.tile([128, SCC * 4], F32, tag="pt", name="pead")
                    for jj in range(nch):
                        qq = s * SCC + jj
                        nc.tensor.matmul(
                            pead[:, jj * 4:(jj + 1) * 4],
                            oht_t[:, jj * 128:(jj + 1) * 128],
                            adts[t_of_q[qq]][:, 0, 8:16].bitcast(F32),
                            start=True, stop=True)
                    asrc = gA[:, 0:nch, HC:HC + 8].bitcast(F32)
                    t0 = work.tile([128, SCC, 4], F32, tag="t0", bufs=2)
                    nc.vector.tensor_add(
                        t0[:, 0:nch, :], asrc, ae_s[:, s * SCC:s * SCC + nch, :])
                    t1 = work.tile([128, SCC, 4], F32, tag="t1", bufs=2)
                    nc.vector.tensor_add(
                        t1[:, 0:nch, :], t0[:, 0:nch, :],
                        pead[:, 0:nch * 4].rearrange("x (a b) -> x a b", b=4))
                    t2 = work.tile([128, SCC, 4], F32, tag="t2", bufs=2)
                    nc.scalar.activation(
                        t2[:, 0:nch, :], t1[:, 0:nch, :], AF.Copy, scale=0.2)
                    t3 = work.tile([128, SCC, 4], F32, tag="t3", bufs=2)
                    nc.vector.tensor_max(
                        t3[:, 0:nch, :], t1[:, 0:nch, :], t2[:, 0:nch, :])
                    p = work.tile([128, SCC, 4], F32, tag="p", bufs=2)
                    nc.scalar.activation(p[:, 0:nch, :], t3[:, 0:nch, :], AF.Exp)
                    pb = work.tile([128, SCC, 4], BF16, tag="pb", bufs=2)
                    nc.scalar.activation(pb[:, 0:nch, :], p[:, 0:nch, :], AF.Copy)
                    return gA, ohb_t, p, pb

                # alpha_dst rows for every dst tile (tiny 128-row gathers)
                for t in range(NT):
                    adt = adp.tile([128, 1, 128], BF16, tag=f"adt{t}",
                                   name=f"adt{t}")
                    nc.gpsimd.dma_gather(
                        adt[:], tbl[:, HC:TW], trw_s[:, t * 8:(t + 1) * 8],
                        128, 128, 128, elem_step=TW, single_packet=False,
                        queue_num=3)
                    adts[t] = adt
                q = 0
                gA = ohb_t = p = pb = None
                for t in range(NT):
                    ps_o = psacc.tile([128, HC], F32, tag="ph")
                    ps_s = pss.tile([128, 8], F32, tag="pss")
                    for k in range(K_t[t]):
                        s, j = divmod(q, SCC)
                        if j == 0:
                            gA, ohb_t, p, pb = ensure_super(s)
                        gp = work.tile([128, HC], BF16, tag="gp", bufs=4)
                        nc.vector.tensor_mul(
                            gp[:].rearrange("x (h c) -> x h c", h=H),
                            gA[:, j, 0:HC].rearrange("x (h c) -> x h c", h=H),
                            _bcast4(p, j, C))
                        first, last = (k == 0), (k == K_t[t] - 1)
                        oh_j = ohb_t[:, j * 128:(j + 1) * 128]
                        nc.tensor.matmul(ps_o[:], oh_j, gp[:], start=first, stop=last)
                        nc.tensor.matmul(ps_s[:, 0:4], oh_j, pb[:, j, :],
                                         start=first, stop=last)
                        q += 1
                    finalize(t, ps_o, ps_s)

            def norm_relu(ps_o, ps_s, bias_s):
                sp = work.tile([128, 4], F32, tag="sp")
                nc.vector.tensor_scalar_add(sp[:], ps_s[:, 0:4], 1e-16)
                rc = work.tile([128, 4], F32, tag="rc")
                nc.vector.reciprocal(rc[:], sp[:])
                ao = work.tile([128, HC], F32, tag="ao", bufs=2)
                for h in range(H):
                    nc.vector.tensor_scalar_mul(
                        ao[:, h * C:(h + 1) * C], ps_o[:, h * C:(h + 1) * C],
                        rc[:, h:h + 1])
                if bias_s is not None:
                    ab = work.tile([128, HC], F32, tag="ao", bufs=2)
                    nc.vector.tensor_add(ab[:], ao[:], bias_s[:])
                    ao = ab
                ar = work.tile([128, HC], F32, tag="ar", bufs=2)
                nc.scalar.activation(ar[:], ao[:], AF.Relu)
                return ar

            # ---- layer-0 finalize: transpose + layer-1 linear + H1 group AG
            def fin0(t, ps_o, ps_s):
                ar = norm_relu(ps_o, ps_s, b0_s if use_b0 else None)
                a0k = []
                for kk in range(KB):
                    pt = pstr.tile([128, 128], F32, tag="pt")
                    nc.tensor.transpose(pt[:], ar[:, kk * 128:(kk + 1) * 128], id_s[:])
                    ak = tp.tile([128, 128], BF16, tag="a1T", name=f"a0k{kk}")
                    nc.vector.tensor_copy(ak[:], pt[:])
                    a0k.append(ak)
                ph1 = psacc.tile([128, HC], F32, tag="ph")
                pa1 = pss.tile([128, 8], F32, tag="pss")
                for kk in range(KB):
                    first, last = (kk == 0), (kk == KB - 1)
                    nc.tensor.matmul(ph1[:], a0k[kk][:], r1h_s[kk][:],
                                     start=first, stop=last)
                    nc.tensor.matmul(pa1[:], a0k[kk][:], r1a_s[kk][:],
                                     start=first, stop=last)
                st = stage.tile([128, TW], BF16, tag="hrow")
                if t % 2 == 0:
                    nc.vector.tensor_copy(st[:, 0:HC], ph1[:])
                else:
                    nc.scalar.activation(st[:, 0:HC], ph1[:], AF.Copy)
                nc.vector.tensor_copy(st[:, HC:HC + 16].bitcast(F32), pa1[:])
                g = t // GPG
                loc = t % GPG
                nc.sync.dma_start(out=H1g[g][loc * 128:(loc + 1) * 128, :], in_=st[:])
                if loc == GPG - 1:
                    nc.gpsimd.collective_compute(
                        "AllGather", OP.bypass,
                        replica_groups=[list(range(NCORES))],
                        ins=[H1g[g].opt()],
                        outs=[H1[g * NCORES * GR:(g + 1) * NCORES * GR, :].opt()],
                    )

            nc._state.push_named_scope("phaseB")
            agg_layer(H0, ae0_s, fin0)
            nc._state.pop_named_scope("phaseB")

            # ---- layer-1 aggregation + MLP head per dst tile ----
            def fin1(t, ps_o, ps_s):
                ar = norm_relu(ps_o, ps_s, b1_s if use_b1 else None)
                h2p = psacc.tile([128, FTS], F32, tag="ph")
                for kk in range(KB):
                    pt = pstr.tile([128, 128], F32, tag="pt")
                    nc.tensor.transpose(pt[:], ar[:, kk * 128:(kk + 1) * 128], id_s[:])
                    a1k = tp.tile([128, 128], F32, tag="a1T")
                    nc.vector.tensor_copy(a1k[:], pt[:])
                    nc.tensor.matmul(h2p[:], a1k[:], r2_s[kk][:],
                                     start=(kk == 0), stop=(kk == KB - 1))
                if use_l0b:
                    h2b = work.tile([128, FTS], F32, tag="h2b")
                    nc.vector.tensor_add(h2b[:], h2p[:], l0b_s[:])
                else:
                    h2b = h2p
                h2r = work.tile([128, FTS], F32, tag="h2r")
                nc.scalar.activation(h2r[:], h2b[:], AF.Relu)
                pt2 = pstr.tile([128, 128], F32, tag="pt")
                nc.tensor.transpose(pt2[:], h2r[:], id_s[:])
                h2T = tp.tile([128, 128], F32, tag="a1T")
                nc.vector.tensor_copy(h2T[:], pt2[:])
                po = pss.tile([128, 8], F32, tag="pss")
                nc.tensor.matmul(po[:, 0:1], h2T[:], r3_s[:], start=True, stop=True)
                ob = work.tile([128, 1], F32, tag="ob")
                if use_l1b:
                    nc.vector.tensor_add(ob[:], po[:, 0:1], l1b_s[:])
                else:
                    nc.vector.tensor_copy(ob[:], po[:, 0:1])
                nc.sync.dma_start(out=outp[t * 128:(t + 1) * 128, :], in_=ob[:])

            nc._state.push_named_scope("phaseD")
            ae1_s = cload(ae1[:, :, :], [128, NCHUNK, 4], tag="ae")
            agg_layer(H1, ae1_s, fin1)
            nc._state.pop_named_scope("phaseD")

    nc.finalize()
    return nc


def _wrap_idx(v, E_pad):
    blk = np.zeros((16, E_pad // 16), np.int16)
    ar = np.arange(E_pad)
    blk[ar % 16, ar // 16] = v.astype(np.int16)
    return np.tile(blk, (8, 1))


def kernel(x, edge_index, edge_weights,
           W0, as0, ad0, We0, ae0, b0,
           W1, as1, ad1, We1, ae1, b1,
           L0W, L0b, L1W, L1b):
    x = np.asarray(x, np.float32)
    N, F_IN = x.shape
    HC = W0.shape[0]
    H, C = np.asarray(as0).shape
    FTS = np.asarray(L0W).shape[0]

    NT = -(-N // (128 * NCORES))
    SHARD = NT * 128
    NP = SHARD * NCORES
    NAG = 1
    for cand in (4, 5, 2):
        if NT % cand == 0:
            NAG = cand
            break
    GPG = NT // NAG
    GR = GPG * 128

    # table-row permutation (group-major) so group AllGathers land contiguous
    nodes = np.arange(NP)
    core = nodes // SHARD
    rr = nodes % SHARD
    gg = rr // GR
    off = rr % GR
    t_of_n = gg * (NCORES * GR) + core * GR + off     # node -> table row

    # ---- edges ----
    ew_in = np.asarray(edge_weights, np.float32)
    src = np.concatenate([np.asarray(edge_index[0]), np.arange(N)])
    dst = np.concatenate([np.asarray(edge_index[1]), np.arange(N)])
    ew = np.concatenate([ew_in, np.full(N, ew_in.mean(), np.float32)])
    order = np.argsort(dst, kind="stable")
    src_s, dst_s, ew_s = src[order], dst[order], ew[order]

    NTG = NP // 128
    tile_of = (dst_s // 128).astype(np.int64)
    tcounts = np.bincount(tile_of, minlength=NTG)
    tstart = np.concatenate([[0], np.cumsum(tcounts)])

    K_t = [max(1, int(max(-(-tcounts[i * NT + t] // 128) for i in range(NCORES))))
           for t in range(NT)]
    NCHUNK = int(sum(K_t))
    E_pad = NCHUNK * 128

    # ---- weight folding (host, O(weights)) ----
    as0 = np.asarray(as0, np.float32)
    ad0 = np.asarray(ad0, np.float32)
    ae0w = np.asarray(ae0, np.float32)
    as1 = np.asarray(as1, np.float32)
    ad1 = np.asarray(ad1, np.float32)
    ae1w = np.asarray(ae1, np.float32)
    W0 = np.asarray(W0, np.float32)
    W1 = np.asarray(W1, np.float32)
    We0 = np.asarray(We0, np.float32)
    We1 = np.asarray(We1, np.float32)

    k0 = (We0.reshape(H, C) * ae0w).sum(1).astype(np.float32)
    k1 = (We1.reshape(H, C) * ae1w).sum(1).astype(np.float32)

    def fold(W, a):
        blk = np.zeros((HC, H), np.float32)
        for h in range(H):
            blk[h * C:(h + 1) * C, h] = a[h]
        return (W.T @ blk).astype(np.float32)

    bf = ml_dtypes.bfloat16
    r0h = W0.T.astype(bf)
    r0a = np.concatenate([fold(W0, as0), fold(W0, ad0)], 1).astype(bf)
    r1h = W1.T.astype(bf)
    r1a = np.concatenate([fold(W1, as1), fold(W1, ad1)], 1).astype(bf)
    r2 = np.asarray(L0W, np.float32).T.copy()
    r3 = np.asarray(L1W, np.float32).T.copy()

    # xT in TABLE-ROW order: column r of xT = x[node(r)]
    inv = np.empty(NP, np.int64)
    inv[t_of_n] = nodes                              # table row -> node
    xa = np.zeros((NP, F_IN), np.float32)
    xa[:N] = x
    xT = np.ascontiguousarray(xa[inv].T).astype(bf)

    b0t = np.tile(np.asarray(b0, np.float32)[None, :], (128, 1))
    b1t = np.tile(np.asarray(b1, np.float32)[None, :], (128, 1))
    l0bt = np.tile(np.asarray(L0b, np.float32)[None, :], (128, 1))
    l1bt = np.tile(np.asarray(L1b, np.float32).reshape(1, 1), (128, 1))
    ident = np.eye(128, dtype=np.float32)

    in_maps = []
    eye128 = np.eye(128, dtype=np.float32)
    for i in range(NCORES):
        srcp = np.zeros(E_pad, np.int64)
        dlocp = np.full(E_pad, -1, np.int64)
        ewp = np.zeros(E_pad, np.float32)
        offq = 0
        for t in range(NT):
            g = i * NT + t
            cnt = int(tcounts[g])
            sl = slice(tstart[g], tstart[g] + cnt)
            srcp[offq:offq + cnt] = t_of_n[src_s[sl]]
            dlocp[offq:offq + cnt] = dst_s[sl] - g * 128
            ewp[offq:offq + cnt] = ew_s[sl]
            offq += K_t[t] * 128
        ae0p = (ewp[:, None] * k0[None, :]).reshape(NCHUNK, 128, 4).transpose(1, 0, 2)
        ae1p = (ewp[:, None] * k1[None, :]).reshape(NCHUNK, 128, 4).transpose(1, 0, 2)
        # one-hot blocks: ohb[e, q*128 + d] / oht[d, q*128 + e]
        ohcube = np.zeros((NCHUNK, 128, 128), np.float32)  # [q, e, d]
        dl2 = dlocp.reshape(NCHUNK, 128)
        valid = dl2 >= 0
        qs, es = np.nonzero(valid)
        ohcube[qs, es, dl2[qs, es]] = 1.0
        ohb_np = np.ascontiguousarray(
            ohcube.transpose(1, 0, 2).reshape(128, NCHUNK * 128)).astype(bf)
        oht_np = np.ascontiguousarray(
            ohcube.transpose(2, 0, 1).reshape(128, NCHUNK * 128))
        # per-tile table rows for the alpha_dst row-gather
        trows = np.empty((NT, 128), np.int64)
        for t in range(NT):
            base = t_of_n[i * SHARD + t * 128]
            trows[t] = base + np.arange(128)
        trw_np = np.concatenate(
            [_wrap_idx(trows[t], 128) for t in range(NT)], axis=1)
        in_maps.append({
            "xT": xT, "r0h": r0h, "r0a": r0a, "r1h": r1h, "r1a": r1a,
            "r2": r2, "r3": r3, "b0t": b0t, "b1t": b1t, "l0bt": l0bt,
            "l1bt": l1bt, "ident": ident,
            "srcw": _wrap_idx(srcp, E_pad), "trw": trw_np,
            "ohb": ohb_np, "oht": oht_np,
            "ae0": np.ascontiguousarray(ae0p),
            "ae1": np.ascontiguousarray(ae1p),
        })

    nc = _build_program(NP, F_IN, HC, H, C, NT, K_t, FTS, NAG,
                        bool(np.any(b0)), bool(np.any(b1)),
                        bool(np.any(np.asarray(L0b))), bool(np.any(np.asarray(L1b))))
    res = run_bass_kernel_spmd(nc, in_maps, list(range(NCORES)))
    out = np.concatenate([res.results[i]["out"][:, 0] for i in range(NCORES)])
    return out[:N].astype(np.float32)



# revision 7
# speedup vs baseline: 1.2118x; 1.2118x over previous
"""2-layer GAT + MLP head on 8 TRN2 NeuronCores.

Strategy (dst-sharded, software-pipelined):
- Nodes padded to NP=20480; each core owns a contiguous 2560-dst shard.
- Edges (incl. self-loops, PyG mean-fill edge attr) sorted by dst,
  grouped into 128-dst tiles, padded per tile-slot to a chunk count K_t
  shared by all cores (SPMD: one program).
- Per layer a node table [NP, 640] bf16 in HBM: cols [0:512) = h,
  bytes [1024:1056) = asrc|adst (f32 bits). Rows permuted group-major
  so layer-1 tables assemble from per-group AllGathers (Shared HBM).
- Aggregation per 128-edge chunk: gather rows by src (1 DMA per
  8-chunk super), one-hot blocks ([e,d] bf16 + [d,e] bf16) streamed as
  one fused ohz tensor; p = exp(lrelu(asrc+adst+aedge)) with adst
  expanded via one-hot matmul; out[dst] += (p*h) via one-hot matmul in
  PSUM; denominator via second matmul with rhs=p.
- Pipelining: gathers issued 2 supers ahead, alpha chains 1 super
  ahead, tile finalize deferred by 1 tile so the in-order PE stream
  never waits on vector/scalar chains.
- dst-tile alphas for layer 1 captured into SBUF during fin0 (no
  gather); layer-0 ones via one batched 2560-row gather from H0.
"""

import numpy as np
import ml_dtypes

import concourse.bacc as bacc
import concourse.bass as bass
import concourse.mybir as mybir
import concourse.tile as tile
from concourse.bass_utils import run_bass_kernel_spmd

F32 = mybir.dt.float32
BF16 = mybir.dt.bfloat16
I16 = mybir.dt.int16
AF = mybir.ActivationFunctionType
OP = mybir.AluOpType

NCORES = 8
SCC = 8  # chunks (of 128 edges) per gather super-chunk


def _bcast4(ap_tile, j, reps):
    """[128, SCC, 4] tile -> [128, 4, reps] zero-step broadcast AP of slot j."""
    sl = ap_tile[:, j, :]
    return bass.AP(sl.tensor, sl.offset, [list(sl.ap[0]), list(sl.ap[-1]), [0, reps]])


def _build_program(NP, F_IN, HC, H, C, NT, K_t, FTS, NAG, use_b0, use_b1,
                   use_l0b, use_l1b):
    NCHUNK = int(sum(K_t))
    E_pad = NCHUNK * 128
    SW = E_pad // 16
    TW = HC + 128  # bf16 table row: h | asrc,adst (f32 bits) | pad
    KB = HC // 128
    GPG = NT // NAG          # dst tiles per allgather group
    GR = GPG * 128           # rows per group per core
    NSUP = -(-NCHUNK // SCC)
    MT = NP // 128

    # chunk q -> owning dst tile
    t_of_q = []
    for t in range(NT):
        t_of_q += [t] * K_t[t]

    nc = bacc.Bacc(dynamic_dma_scratch_size=65536, num_swdge_queues=4)
    P = nc.declare_dram_parameter

    xT = P("xT", [F_IN, NP], BF16, isOutput=False)
    r0h = P("r0h", [F_IN, HC], BF16, isOutput=False)
    r0a = P("r0a", [F_IN, 8], BF16, isOutput=False)
    r1h = P("r1h", [HC, HC], BF16, isOutput=False)
    r1a = P("r1a", [HC, 8], BF16, isOutput=False)
    r2 = P("r2", [HC, FTS], F32, isOutput=False)
    r3 = P("r3", [FTS, 1], F32, isOutput=False)
    ident = P("ident", [128, 128], F32, isOutput=False)
    identb = P("identb", [128, 128], BF16, isOutput=False)
    srcw = P("srcw", [128, SW], I16, isOutput=False)
    trw = P("trw", [128, NT * 8], I16, isOutput=False)
    ohz = P("ohz", [128, NCHUNK, 256], BF16, isOutput=False)
    ae0 = P("ae0", [128, NCHUNK, 4], F32, isOutput=False)
    ae1 = P("ae1", [128, NCHUNK, 4], F32, isOutput=False)
    if use_b0:
        b0t = P("b0t", [128, HC], F32, isOutput=False)
    if use_b1:
        b1t = P("b1t", [128, HC], F32, isOutput=False)
    if use_l0b:
        l0bt = P("l0bt", [128, FTS], F32, isOutput=False)
    if use_l1b:
        l1bt = P("l1bt", [128, 1], F32, isOutput=False)
    outp = P("out", [NT * 128, 1], F32, isOutput=True)

    with tile.TileContext(nc) as tc:
        with (
            tc.tile_pool(name="const", bufs=1) as const,
            tc.tile_pool(name="stage", bufs=2) as stage,
            tc.tile_pool(name="work", bufs=3) as work,
            tc.tile_pool(name="tp", bufs=6) as tp,
            tc.tile_pool(name="adp", bufs=1) as adp,
            tc.tile_pool(name="psacc", bufs=3, space="PSUM") as psacc,
            tc.tile_pool(name="pss", bufs=2, space="PSUM") as pss,
            tc.tile_pool(name="pstr", bufs=3, space="PSUM") as pstr,
            tc.tile_pool(name="dram", bufs=1, space="DRAM") as dram,
        ):
            H0 = dram.tile([NP, TW], BF16, tag="H0")
            H1 = dram.tile([NP, TW], BF16, tag="H1")
            H1g = [dram.tile([GR, TW], BF16, tag=f"H1g{g}", name=f"H1g{g}")
                   for g in range(NAG)]

            _cn = [0]

            def cload(ap_in, shape, dt=F32, tag=None):
                _cn[0] += 1
                cname = tag or f"c{_cn[0]}"
                t = const.tile(shape, dt, tag=cname, name=f"{cname}_{_cn[0]}")
                nc.sync.dma_start(out=t[:], in_=ap_in)
                return t

            r0h_s = cload(r0h[:, :], [F_IN, HC], BF16)
            r0a_s = cload(r0a[:, :], [F_IN, 8], BF16)
            r1h_s = [cload(r1h[k * 128:(k + 1) * 128, :], [128, HC], BF16)
                     for k in range(KB)]
            r1a_s = [cload(r1a[k * 128:(k + 1) * 128, :], [128, 8], BF16)
                     for k in range(KB)]
            r2_s = [cload(r2[k * 128:(k + 1) * 128, :], [128, FTS]) for k in range(KB)]
            r3_s = cload(r3[:, :], [FTS, 1])
            id_s = cload(ident[:, :], [128, 128])
            idb_s = cload(identb[:, :], [128, 128], BF16)
            srcw_s = cload(srcw[:, :], [128, SW], I16)
            trw_s = cload(trw[:, :], [128, NT * 8], I16)
            ae0_s = cload(ae0[:, :, :], [128, NCHUNK, 4], tag="ae")
            b0_s = cload(b0t[:, :], [128, HC]) if use_b0 else None
            b1_s = cload(b1t[:, :], [128, HC]) if use_b1 else None
            l0b_s = cload(l0bt[:, :], [128, FTS]) if use_l0b else None
            l1b_s = cload(l1bt[:, :], [128, 1]) if use_l1b else None
            xTs = cload(xT[:, :], [F_IN, NP], BF16, tag="xT")

            # persistent alpha_dst for layer-1's own dst tiles (filled in fin0)
            adts1 = adp.tile([128, NT, 4], BF16, tag="adts1")

            # ---- phase A: layer-0 table (xT pre-permuted to row order) ----
            nc._state.push_named_scope("phaseA")
            for mr in range(MT):
                lx = xTs[:, mr * 128:(mr + 1) * 128]
                ph = psacc.tile([128, HC], F32, tag="ph")
                nc.tensor.matmul(ph[:], lx, r0h_s[:], start=True, stop=True)
                pa = pss.tile([128, 8], F32, tag="pss")
                nc.tensor.matmul(pa[:], lx, r0a_s[:], start=True, stop=True)
                st = stage.tile([128, TW], BF16, tag="hrow", bufs=4)
                if mr % 2 == 0:
                    nc.vector.tensor_copy(st[:, 0:HC], ph[:])
                else:
                    nc.scalar.activation(st[:, 0:HC], ph[:], AF.Copy)
                nc.vector.tensor_copy(st[:, HC:HC + 16].bitcast(F32), pa[:])
                nc.sync.dma_start(out=H0[mr * 128:(mr + 1) * 128, :], in_=st[:])
            nc._state.pop_named_scope("phaseA")

            # ---- layer-0 alpha_dst for own dst tiles: one batched gather ----
            adt_all = adp.tile([128, NT, 128], BF16, tag="adta")
            nc.gpsimd.dma_gather(
                adt_all[:], H0[:, HC:TW], trw_s[:, 0:NT * 8],
                NT * 128, NT * 128, 128, elem_step=TW,
                single_packet=False, queue_num=3)
            adts0 = adp.tile([128, NT, 4], BF16, tag="adts0")
            nc.vector.tensor_copy(adts0[:], adt_all[:, :, 8:16].bitcast(F32))

            # ---- aggregation over one layer's edges ----
            def agg_layer(tbl, ae_s, adts, fin_pre, fin_post):
                gstate = {}

                def nch_of(s):
                    return min(SCC, NCHUNK - s * SCC)

                def issue_gather(s):
                    if s >= NSUP:
                        return
                    nch = nch_of(s)
                    gA = stage.tile([128, SCC, TW], BF16, tag="gA",
                                    name="gA", bufs=3)
                    nc.gpsimd.dma_gather(
                        gA[:, 0:nch, :], tbl[:, :],
                        srcw_s[:, s * SCC * 8:(s * SCC + nch) * 8],
                        nch * 128, nch * 128, TW,
                        single_packet=False, queue_num=s % 4)
                    oz = stage.tile([128, SCC, 256], BF16, tag="ohz",
                                    name="oz", bufs=4)
                    nc.sync.dma_start(
                        out=oz[:, 0:nch, :],
                        in_=ohz[:, s * SCC:s * SCC + nch, :])
                    gstate[s] = [gA, oz, None, None]

                def emit_pead(s):
                    if s >= NSUP:
                        return
                    nch = nch_of(s)
                    oz = gstate[s][1]
                    pead = pstr.tile([128, SCC * 4], F32, tag="pt", name="pead")
                    for jj in range(nch):
                        q = s * SCC + jj
                        nc.tensor.matmul(
                            pead[:, jj * 4:(jj + 1) * 4],
                            oz[:, jj, 128:256], adts[:, t_of_q[q], :],
                            start=True, stop=True)
                    gstate[s][2] = pead

                def emit_alpha(s):
                    if s >= NSUP:
                        return
                    nch = nch_of(s)
                    gA, oz, pead, _ = gstate[s]
                    asrc = gA[:, 0:nch, HC:HC + 8].bitcast(F32)
                    t0 = work.tile([128, SCC, 4], F32, tag="t0", bufs=2)
                    nc.vector.tensor_add(
                        t0[:, 0:nch, :], asrc,
                        ae_s[:, s * SCC:s * SCC + nch, :])
                    t1 = work.tile([128, SCC, 4], F32, tag="t1", bufs=2)
                    nc.vector.tensor_add(
                        t1[:, 0:nch, :], t0[:, 0:nch, :],
                        pead[:, 0:nch * 4].rearrange("x (a b) -> x a b", b=4))
                    t2 = work.tile([128, SCC, 4], F32, tag="t2", bufs=2)
                    nc.scalar.activation(
                        t2[:, 0:nch, :], t1[:, 0:nch, :], AF.Copy, scale=0.2)
                    tl = work.tile([128, SCC, 4], F32, tag="tl", bufs=2)
                    nc.vector.tensor_max(
                        tl[:, 0:nch, :], t1[:, 0:nch, :], t2[:, 0:nch, :])
                    p = work.tile([128, SCC, 4], BF16, tag="p", bufs=3)
                    nc.scalar.activation(p[:, 0:nch, :], tl[:, 0:nch, :], AF.Exp)
                    gstate[s][3] = p

                issue_gather(0)
                issue_gather(1)
                emit_pead(0)
                emit_alpha(0)

                prev = None
                q = 0
                for t in range(NT):
                    ps_o = psacc.tile([128, HC], F32, tag="ph")
                    ps_s = pss.tile([128, 8], F32, tag="pss")
                    for k in range(K_t[t]):
                        s, j = divmod(q, SCC)
                        if j == 0:
                            issue_gather(s + 2)
                            emit_pead(s + 1)
                        if j == 2:
                            emit_alpha(s + 1)
                        gA, oz, pead, p = gstate[s]
                        gp = work.tile([128, HC], BF16, tag="gp", bufs=4)
                        nc.vector.tensor_mul(
                            gp[:].rearrange("x (h c) -> x h c", h=H),
                            gA[:, j, 0:HC].rearrange("x (h c) -> x h c", h=H),
                            _bcast4(p, j, C))
                        first, last = (k == 0), (k == K_t[t] - 1)
                        oh_j = oz[:, j, 0:128]
                        nc.tensor.matmul(ps_o[:], oh_j, gp[:],
                                         start=first, stop=last)
                        nc.tensor.matmul(ps_s[:, 0:4], oh_j, p[:, j, :],
                                         start=first, stop=last)
                        q += 1
                    ar = fin_pre(t, ps_o, ps_s)
                    if prev is not None:
                        fin_post(*prev)
                    prev = (t, ar)
                fin_post(*prev)

            # ---- tile finalize: softmax-normalize + relu (fused on scalar) --
            def norm_relu(ps_o, ps_s, bias_s, out_dt):
                sp = work.tile([128, 4], F32, tag="sp")
                nc.vector.tensor_scalar_add(sp[:], ps_s[:, 0:4], 1e-16)
                rc = work.tile([128, 4], F32, tag="rc")
                nc.vector.reciprocal(rc[:], sp[:])
                if bias_s is None:
                    ar = work.tile([128, HC], out_dt,
                                   tag=f"ar{out_dt}", bufs=3)
                    for h in range(H):
                        nc.scalar.activation(
                            ar[:, h * C:(h + 1) * C], ps_o[:, h * C:(h + 1) * C],
                            AF.Relu, scale=rc[:, h:h + 1])
                    return ar
                ao = work.tile([128, HC], F32, tag="ao", bufs=2)
                for h in range(H):
                    nc.vector.tensor_scalar_mul(
                        ao[:, h * C:(h + 1) * C], ps_o[:, h * C:(h + 1) * C],
                        rc[:, h:h + 1])
                ab = work.tile([128, HC], F32, tag="ao", bufs=2)
                nc.vector.tensor_add(ab[:], ao[:], bias_s[:])
                ar = work.tile([128, HC], out_dt, tag=f"ar{out_dt}", bufs=3)
                nc.scalar.activation(ar[:], ab[:], AF.Relu)
                return ar

            # ---- layer-0 finalize: transpose + layer-1 linear + group AG ----
            def fin_pre0(t, ps_o, ps_s):
                return norm_relu(ps_o, ps_s, b0_s, BF16)

            def fin_post0(t, ar):
                a0k = []
                for kk in range(KB):
                    pt = pstr.tile([128, 128], BF16, tag="pt", name="ptb")
                    nc.tensor.transpose(pt[:], ar[:, kk * 128:(kk + 1) * 128],
                                        idb_s[:])
                    ak = tp.tile([128, 128], BF16, tag="a1T", name=f"a0k{kk}")
                    nc.vector.tensor_copy(ak[:], pt[:])
                    a0k.append(ak)
                ph1 = psacc.tile([128, HC], F32, tag="ph")
                pa1 = pss.tile([128, 8], F32, tag="pss")
                for kk in range(KB):
                    first, last = (kk == 0), (kk == KB - 1)
                    nc.tensor.matmul(ph1[:], a0k[kk][:], r1h_s[kk][:],
                                     start=first, stop=last)
                    nc.tensor.matmul(pa1[:], a0k[kk][:], r1a_s[kk][:],
                                     start=first, stop=last)
                st = stage.tile([128, TW], BF16, tag="hrow", bufs=4)
                if t % 2 == 0:
                    nc.vector.tensor_copy(st[:, 0:HC], ph1[:])
                else:
                    nc.scalar.activation(st[:, 0:HC], ph1[:], AF.Copy)
                nc.vector.tensor_copy(st[:, HC:HC + 16].bitcast(F32), pa1[:])
                nc.vector.tensor_copy(adts1[:, t, :], pa1[:, 4:8])
                g = t // GPG
                loc = t % GPG
                nc.sync.dma_start(out=H1g[g][loc * 128:(loc + 1) * 128, :],
                                  in_=st[:])
                if loc == GPG - 1:
                    nc.gpsimd.collective_compute(
                        "AllGather", OP.bypass,
                        replica_groups=[list(range(NCORES))],
                        ins=[H1g[g].opt()],
                        outs=[H1[g * NCORES * GR:(g + 1) * NCORES * GR, :].opt()],
                    )

            nc._state.push_named_scope("phaseB")
            agg_layer(H0, ae0_s, adts0, fin_pre0, fin_post0)
            nc._state.pop_named_scope("phaseB")

            # ---- layer-1 aggregation + MLP head per dst tile ----
            def fin_pre1(t, ps_o, ps_s):
                return norm_relu(ps_o, ps_s, b1_s, F32)

            def fin_post1(t, ar):
                h2p = psacc.tile([128, FTS], F32, tag="ph")
                for kk in range(KB):
                    pt = pstr.tile([128, 128], F32, tag="pt", name="ptf")
                    nc.tensor.transpose(pt[:], ar[:, kk * 128:(kk + 1) * 128],
                                        id_s[:])
                    a1k = tp.tile([128, 128], F32, tag="a1T")
                    nc.vector.tensor_copy(a1k[:], pt[:])
                    nc.tensor.matmul(h2p[:], a1k[:], r2_s[kk][:],
                                     start=(kk == 0), stop=(kk == KB - 1))
                if use_l0b:
                    h2b = work.tile([128, FTS], F32, tag="h2b")
                    nc.vector.tensor_add(h2b[:], h2p[:], l0b_s[:])
                else:
                    h2b = h2p
                h2r = work.tile([128, FTS], F32, tag="h2r")
                nc.scalar.activation(h2r[:], h2b[:], AF.Relu)
                pt2 = pstr.tile([128, 128], F32, tag="pt")
                nc.tensor.transpose(pt2[:], h2r[:], id_s[:])
                h2T = tp.tile([128, 128], F32, tag="a1T")
                nc.vector.tensor_copy(h2T[:], pt2[:])
                po = pss.tile([128, 8], F32, tag="pss")
                nc.tensor.matmul(po[:, 0:1], h2T[:], r3_s[:],
                                 start=True, stop=True)
                ob = work.tile([128, 1], F32, tag="ob")
                if use_l1b:
                    nc.vector.tensor_add(ob[:], po[:, 0:1], l1b_s[:])
                else:
                    nc.vector.tensor_copy(ob[:], po[:, 0:1])
                nc.sync.dma_start(out=outp[t * 128:(t + 1) * 128, :], in_=ob[:])

            nc._state.push_named_scope("phaseD")
            ae1_s = cload(ae1[:, :, :], [128, NCHUNK, 4], tag="ae")
            agg_layer(H1, ae1_s, adts1, fin_pre1, fin_post1)
            nc._state.pop_named_scope("phaseD")

    nc.finalize()
    return nc


def _wrap_idx(v, E_pad):
    blk = np.zeros((16, E_pad // 16), np.int16)
    ar = np.arange(E_pad)
    blk[ar % 16, ar // 16] = v.astype(np.int16)
    return np.tile(blk, (8, 1))


def kernel(x, edge_index, edge_weights,
           W0, as0, ad0, We0, ae0, b0,
           W1, as1, ad1, We1, ae1, b1,
           L0W, L0b, L1W, L1b):
    x = np.asarray(x, np.float32)
    N, F_IN = x.shape
    HC = W0.shape[0]
    H, C = np.asarray(as0).shape
    FTS = np.asarray(L0W).shape[0]

    NT = -(-N // (128 * NCORES))
    SHARD = NT * 128
    NP = SHARD * NCORES
    NAG = NT // 2 if NT % 2 == 0 else NT
    GPG = NT // NAG
    GR = GPG * 128

    # table-row permutation (group-major) so group AllGathers land contiguous
    nodes = np.arange(NP)
    core = nodes // SHARD
    rr = nodes % SHARD
    gg = rr // GR
    off = rr % GR
    t_of_n = gg * (NCORES * GR) + core * GR + off     # node -> table row

    # ---- edges ----
    ew_in = np.asarray(edge_weights, np.float32)
    src = np.concatenate([np.asarray(edge_index[0]), np.arange(N)])
    dst = np.concatenate([np.asarray(edge_index[1]), np.arange(N)])
    ew = np.concatenate([ew_in, np.full(N, ew_in.mean(), np.float32)])
    order = np.argsort(dst, kind="stable")
    src_s, dst_s, ew_s = src[order], dst[order], ew[order]

    NTG = NP // 128
    tile_of = (dst_s // 128).astype(np.int64)
    tcounts = np.bincount(tile_of, minlength=NTG)
    tstart = np.concatenate([[0], np.cumsum(tcounts)])

    K_t = [max(1, int(max(-(-tcounts[i * NT + t] // 128) for i in range(NCORES))))
           for t in range(NT)]
    NCHUNK = int(sum(K_t))
    E_pad = NCHUNK * 128

    # ---- weight folding (host, O(weights)) ----
    as0 = np.asarray(as0, np.float32)
    ad0 = np.asarray(ad0, np.float32)
    ae0w = np.asarray(ae0, np.float32)
    as1 = np.asarray(as1, np.float32)
    ad1 = np.asarray(ad1, np.float32)
    ae1w = np.asarray(ae1, np.float32)
    W0 = np.asarray(W0, np.float32)
    W1 = np.asarray(W1, np.float32)
    We0 = np.asarray(We0, np.float32)
    We1 = np.asarray(We1, np.float32)

    k0 = (We0.reshape(H, C) * ae0w).sum(1).astype(np.float32)
    k1 = (We1.reshape(H, C) * ae1w).sum(1).astype(np.float32)

    def fold(W, a):
        blk = np.zeros((HC, H), np.float32)
        for h in range(H):
            blk[h * C:(h + 1) * C, h] = a[h]
        return (W.T @ blk).astype(np.float32)

    bf = ml_dtypes.bfloat16
    r0h = W0.T.astype(bf)
    r0a = np.concatenate([fold(W0, as0), fold(W0, ad0)], 1).astype(bf)
    r1h = W1.T.astype(bf)
    r1a = np.concatenate([fold(W1, as1), fold(W1, ad1)], 1).astype(bf)
    r2 = np.asarray(L0W, np.float32).T.copy()
    r3 = np.asarray(L1W, np.float32).T.copy()

    # xT in TABLE-ROW order: column r of xT = x[node(r)]
    inv = np.empty(NP, np.int64)
    inv[t_of_n] = nodes                              # table row -> node
    xa = np.zeros((NP, F_IN), np.float32)
    xa[:N] = x
    xT = np.ascontiguousarray(xa[inv].T).astype(bf)

    ident = np.eye(128, dtype=np.float32)
    identb = np.eye(128, dtype=np.float32).astype(bf)

    use_b0 = bool(np.any(b0))
    use_b1 = bool(np.any(b1))
    use_l0b = bool(np.any(np.asarray(L0b)))
    use_l1b = bool(np.any(np.asarray(L1b)))

    in_maps = []
    for i in range(NCORES):
        srcp = np.zeros(E_pad, np.int64)
        dlocp = np.full(E_pad, -1, np.int64)
        ewp = np.zeros(E_pad, np.float32)
        offq = 0
        for t in range(NT):
            g = i * NT + t
            cnt = int(tcounts[g])
            sl = slice(tstart[g], tstart[g] + cnt)
            srcp[offq:offq + cnt] = t_of_n[src_s[sl]]
            dlocp[offq:offq + cnt] = dst_s[sl] - g * 128
            ewp[offq:offq + cnt] = ew_s[sl]
            offq += K_t[t] * 128
        ae0p = (ewp[:, None] * k0[None, :]).reshape(NCHUNK, 128, 4).transpose(1, 0, 2)
        ae1p = (ewp[:, None] * k1[None, :]).reshape(NCHUNK, 128, 4).transpose(1, 0, 2)
        # one-hot blocks: ohz[e, q, d] = ohb, ohz[d, q, 128+e] = oht
        ohcube = np.zeros((NCHUNK, 128, 128), np.float32)  # [q, e, d]
        dl2 = dlocp.reshape(NCHUNK, 128)
        valid = dl2 >= 0
        qs, es = np.nonzero(valid)
        ohcube[qs, es, dl2[qs, es]] = 1.0
        ohz_np = np.empty((128, NCHUNK, 256), bf)
        ohz_np[:, :, 0:128] = ohcube.transpose(1, 0, 2).astype(bf)
        ohz_np[:, :, 128:256] = ohcube.transpose(2, 0, 1).astype(bf)
        # own dst-tile table rows for the layer-0 alpha_dst gather
        trows = np.empty((NT, 128), np.int64)
        for t in range(NT):
            base = t_of_n[i * SHARD + t * 128]
            trows[t] = base + np.arange(128)
        trw_np = _wrap_idx(trows.reshape(-1), NT * 128)
        im = {
            "xT": xT, "r0h": r0h, "r0a": r0a, "r1h": r1h, "r1a": r1a,
            "r2": r2, "r3": r3, "ident": ident, "identb": identb,
            "srcw": _wrap_idx(srcp, E_pad), "trw": trw_np,
            "ohz": ohz_np,
            "ae0": np.ascontiguousarray(ae0p),
            "ae1": np.ascontiguousarray(ae1p),
        }
        if use_b0:
            im["b0t"] = np.tile(np.asarray(b0, np.float32)[None, :], (128, 1))
        if use_b1:
            im["b1t"] = np.tile(np.asarray(b1, np.float32)[None, :], (128, 1))
        if use_l0b:
            im["l0bt"] = np.tile(np.asarray(L0b, np.float32)[None, :], (128, 1))
        if use_l1b:
            im["l1bt"] = np.tile(np.asarray(L1b, np.float32).reshape(1, 1), (128, 1))
        in_maps.append(im)

    nc = _build_program(NP, F_IN, HC, H, C, NT, K_t, FTS, NAG,
                        use_b0, use_b1, use_l0b, use_l1b)
    res = run_bass_kernel_spmd(nc, in_maps, list(range(NCORES)))
    out = np.concatenate([res.results[i]["out"][:, 0] for i in range(NCORES)])
    return out[:N].astype(np.float32)


# revision 13
# speedup vs baseline: 1.3503x; 1.1144x over previous
"""2-layer GAT + MLP head on 8 TRN2 NeuronCores.

Strategy (dst-sharded, software-pipelined):
- Nodes padded to NP=20480; each core owns a contiguous 2560-dst shard.
- Edges (incl. self-loops, PyG mean-fill edge attr) sorted by dst,
  grouped into 128-dst tiles, padded per tile-slot to a chunk count K_t
  shared by all cores (SPMD: one program).
- Per layer a node table [NP, 640] bf16 in HBM: cols [0:512) = h,
  bytes [1024:1056) = asrc|adst (f32 bits). Rows permuted group-major
  so layer-1 tables assemble from per-group AllGathers (Shared HBM).
- Aggregation per 128-edge chunk: gather rows by src (1 DMA per
  8-chunk super), one-hot blocks ([e,d] bf16 + [d,e] bf16) streamed as
  one fused ohz tensor; p = exp(lrelu(asrc+adst+aedge)) with adst
  expanded via one-hot matmul; out[dst] += (p*h) via one-hot matmul in
  PSUM; denominator via second matmul with rhs=p.
- Pipelining: gathers issued 2 supers ahead, alpha chains 1 super
  ahead, tile finalize deferred by 1 tile so the in-order PE stream
  never waits on vector/scalar chains.
- dst-tile alphas for layer 1 captured into SBUF during fin0 (no
  gather); layer-0 ones via one batched 2560-row gather from H0.
"""

import numpy as np
import ml_dtypes

import concourse.bacc as bacc
import concourse.bass as bass
import concourse.mybir as mybir
import concourse.tile as tile
from concourse.bass_utils import run_bass_kernel_spmd

F32 = mybir.dt.float32
BF16 = mybir.dt.bfloat16
I16 = mybir.dt.int16
AF = mybir.ActivationFunctionType
OP = mybir.AluOpType

NCORES = 8
SCC = 8  # chunks (of 128 edges) per gather super-chunk


def _bcast4(ap_tile, j, reps):
    """[128, SCC, 4] tile -> [128, 4, reps] zero-step broadcast AP of slot j."""
    sl = ap_tile[:, j, :]
    return bass.AP(sl.tensor, sl.offset, [list(sl.ap[0]), list(sl.ap[-1]), [0, reps]])


def _build_program(NP, F_IN, HC, H, C, NT, K_t, FTS, NAG, use_b0, use_b1,
                   use_l0b, use_l1b):
    NCHUNK = int(sum(K_t))
    E_pad = NCHUNK * 128
    SW = E_pad // 16
    TW = HC + 128  # bf16 table row: h | asrc,adst (f32 bits) | pad
    KB = HC // 128
    GPG = NT // NAG          # dst tiles per allgather group
    GR = GPG * 128           # rows per group per core
    NSUP = -(-NCHUNK // SCC)
    MT = NP // 128

    # chunk q -> owning dst tile
    t_of_q = []
    for t in range(NT):
        t_of_q += [t] * K_t[t]

    nc = bacc.Bacc(dynamic_dma_scratch_size=65536, num_swdge_queues=4)
    P = nc.declare_dram_parameter

    xT = P("xT", [F_IN, NP], BF16, isOutput=False)
    r0h = P("r0h", [F_IN, HC], BF16, isOutput=False)
    r0a = P("r0a", [F_IN, 8], BF16, isOutput=False)
    r1h = P("r1h", [HC, HC], BF16, isOutput=False)
    r1a = P("r1a", [HC, 8], BF16, isOutput=False)
    r2 = P("r2", [HC, FTS], F32, isOutput=False)
    r3 = P("r3", [FTS, 1], F32, isOutput=False)
    ident = P("ident", [128, 128], F32, isOutput=False)
    identb = P("identb", [128, 128], BF16, isOutput=False)
    srcw = P("srcw", [128, SW], I16, isOutput=False)
    trw = P("trw", [128, NT * 8], I16, isOutput=False)
    ohz = P("ohz", [128, NCHUNK, 256], BF16, isOutput=False)
    ae0 = P("ae0", [128, NCHUNK, 4], F32, isOutput=False)
    ae1 = P("ae1", [128, NCHUNK, 4], F32, isOutput=False)
    if use_b0:
        b0t = P("b0t", [128, HC], F32, isOutput=False)
    if use_b1:
        b1t = P("b1t", [128, HC], F32, isOutput=False)
    if use_l0b:
        l0bt = P("l0bt", [128, FTS], F32, isOutput=False)
    if use_l1b:
        l1bt = P("l1bt", [128, 1], F32, isOutput=False)
    outp = P("out", [NT * 128, 1], F32, isOutput=True)

    with tile.TileContext(nc) as tc:
        with (
            tc.tile_pool(name="const", bufs=1) as const,
            tc.tile_pool(name="stage", bufs=2) as stage,
            tc.tile_pool(name="work", bufs=3) as work,
            tc.tile_pool(name="tp", bufs=6) as tp,
            tc.tile_pool(name="adp", bufs=1) as adp,
            tc.tile_pool(name="psacc", bufs=3, space="PSUM") as psacc,
            tc.tile_pool(name="pss", bufs=2, space="PSUM") as pss,
            tc.tile_pool(name="pstr", bufs=3, space="PSUM") as pstr,
            tc.tile_pool(name="dram", bufs=1, space="DRAM") as dram,
        ):
            H0 = dram.tile([NP, TW], BF16, tag="H0")
            H1 = dram.tile([NP, TW], BF16, tag="H1")
            H1g = [dram.tile([GR, TW], BF16, tag=f"H1g{g}", name=f"H1g{g}")
                   for g in range(NAG)]

            _cn = [0]

            def cload(ap_in, shape, dt=F32, tag=None):
                _cn[0] += 1
                cname = tag or f"c{_cn[0]}"
                t = const.tile(shape, dt, tag=cname, name=f"{cname}_{_cn[0]}")
                nc.sync.dma_start(out=t[:], in_=ap_in)
                return t

            r0h_s = cload(r0h[:, :], [F_IN, HC], BF16)
            r0a_s = cload(r0a[:, :], [F_IN, 8], BF16)
            r1h_s = [cload(r1h[k * 128:(k + 1) * 128, :], [128, HC], BF16)
                     for k in range(KB)]
            r1a_s = [cload(r1a[k * 128:(k + 1) * 128, :], [128, 8], BF16)
                     for k in range(KB)]
            r2_s = [cload(r2[k * 128:(k + 1) * 128, :], [128, FTS]) for k in range(KB)]
            r3_s = cload(r3[:, :], [FTS, 1])
            id_s = cload(ident[:, :], [128, 128])
            idb_s = cload(identb[:, :], [128, 128], BF16)
            srcw_s = cload(srcw[:, :], [128, SW], I16)
            trw_s = cload(trw[:, :], [128, NT * 8], I16)
            ae0_s = cload(ae0[:, :, :], [128, NCHUNK, 4], tag="ae")
            b0_s = cload(b0t[:, :], [128, HC]) if use_b0 else None
            b1_s = cload(b1t[:, :], [128, HC]) if use_b1 else None
            l0b_s = cload(l0bt[:, :], [128, FTS]) if use_l0b else None
            l1b_s = cload(l1bt[:, :], [128, 1]) if use_l1b else None
            xTs = cload(xT[:, :], [F_IN, NP], BF16, tag="xT")

            # persistent alpha_dst for layer-1's own dst tiles (filled in fin0)
            adts1 = adp.tile([128, NT, 4], BF16, tag="adts1")

            # ---- phase A: layer-0 table (xT pre-permuted to row order) ----
            # PSUM alternates between psacc and pstr pools so the PE streams
            # continuously; stores batched in pairs across sync+scalar queues.
            nc._state.push_named_scope("phaseA")
            st2 = None
            for mr in range(MT):
                lx = xTs[:, mr * 128:(mr + 1) * 128]
                pool = psacc if mr % 2 == 0 else pstr
                ph = pool.tile([128, HC], F32, tag="ph" if mr % 2 == 0 else "pt",
                               name="phA")
                nc.tensor.matmul(ph[:], lx, r0h_s[:], start=True, stop=True)
                pa = pss.tile([128, 8], F32, tag="pss")
                nc.tensor.matmul(pa[:], lx, r0a_s[:], start=True, stop=True)
                half = mr % 2
                if half == 0:
                    st2 = stage.tile([128, 2, TW], BF16, tag="hrow2", bufs=4)
                if mr % 2 == 0:
                    nc.vector.tensor_copy(st2[:, half, 0:HC], ph[:])
                    nc.vector.tensor_copy(
                        st2[:, half, HC:HC + 16].bitcast(F32), pa[:])
                else:
                    nc.scalar.activation(st2[:, half, 0:HC], ph[:], AF.Copy)
                    nc.vector.tensor_copy(
                        st2[:, half, HC:HC + 16].bitcast(F32), pa[:])
                if half == 1:
                    dst = H0[(mr - 1) * 128:(mr + 1) * 128, :].rearrange(
                        "(j p) c -> p j c", p=128)
                    eng = nc.sync if (mr // 2) % 2 == 0 else nc.scalar
                    eng.dma_start(out=dst, in_=st2[:])
            nc._state.pop_named_scope("phaseA")

            # ---- layer-0 alpha_dst for own dst tiles: one batched gather ----
            adt_all = adp.tile([128, NT, 128], BF16, tag="adta")
            nc.gpsimd.dma_gather(
                adt_all[:], H0[:, HC:TW], trw_s[:, 0:NT * 8],
                NT * 128, NT * 128, 128, elem_step=TW,
                single_packet=False, queue_num=3)
            adts0 = adp.tile([128, NT, 4], BF16, tag="adts0")
            nc.vector.tensor_copy(adts0[:], adt_all[:, :, 8:16].bitcast(F32))

            # ---- aggregation over one layer's edges ----
            def agg_layer(tbl, ae_s, adts, fin_pre, fin_post):
                gstate = {}

                def nch_of(s):
                    return min(SCC, NCHUNK - s * SCC)

                def issue_gather(s):
                    if s >= NSUP:
                        return
                    nch = nch_of(s)
                    gA = stage.tile([128, SCC, TW], BF16, tag="gA",
                                    name="gA", bufs=3)
                    c0 = s * SCC * 8
                    base = 0
                    nq = min(4, nch)
                    for qi in range(nq):
                        take = (nch - base + (nq - qi) - 1) // (nq - qi)
                        nc.gpsimd.dma_gather(
                            gA[:, base:base + take, :], tbl[:, :],
                            srcw_s[:, c0 + base * 8:c0 + (base + take) * 8],
                            take * 128, take * 128, TW,
                            single_packet=False, queue_num=qi)
                        base += take
                    oz = stage.tile([128, SCC, 256], BF16, tag="ohz",
                                    name="oz", bufs=4)
                    nc.sync.dma_start(
                        out=oz[:, 0:nch, :],
                        in_=ohz[:, s * SCC:s * SCC + nch, :])
                    gstate[s] = [gA, oz, None, None]

                def emit_pead(s):
                    if s >= NSUP:
                        return
                    nch = nch_of(s)
                    oz = gstate[s][1]
                    pead = pstr.tile([128, SCC * 4], F32, tag="pt", name="pead")
                    for jj in range(nch):
                        q = s * SCC + jj
                        nc.tensor.matmul(
                            pead[:, jj * 4:(jj + 1) * 4],
                            oz[:, jj, 128:256], adts[:, t_of_q[q], :],
                            start=True, stop=True)
                    gstate[s][2] = pead

                def emit_alpha(s):
                    if s >= NSUP:
                        return
                    nch = nch_of(s)
                    gA, oz, pead, _ = gstate[s]
                    asrc = gA[:, 0:nch, HC:HC + 8].bitcast(F32)
                    t0 = work.tile([128, SCC, 4], F32, tag="t0", bufs=2)
                    nc.vector.tensor_add(
                        t0[:, 0:nch, :], asrc,
                        ae_s[:, s * SCC:s * SCC + nch, :])
                    t1 = work.tile([128, SCC, 4], F32, tag="t1", bufs=2)
                    nc.vector.tensor_add(
                        t1[:, 0:nch, :], t0[:, 0:nch, :],
                        pead[:, 0:nch * 4].rearrange("x (a b) -> x a b", b=4))
                    t2 = work.tile([128, SCC, 4], F32, tag="t2", bufs=2)
                    nc.scalar.activation(
                        t2[:, 0:nch, :], t1[:, 0:nch, :], AF.Copy, scale=0.2)
                    tl = work.tile([128, SCC, 4], F32, tag="tl", bufs=2)
                    nc.vector.tensor_max(
                        tl[:, 0:nch, :], t1[:, 0:nch, :], t2[:, 0:nch, :])
                    pf = work.tile([128, SCC, 4], F32, tag="pf", bufs=3)
                    nc.scalar.activation(pf[:, 0:nch, :], tl[:, 0:nch, :], AF.Exp)
                    pb = work.tile([128, SCC, 4], BF16, tag="p", bufs=3)
                    nc.vector.tensor_copy(pb[:, 0:nch, :], pf[:, 0:nch, :])
                    gstate[s][3] = (pf, pb)

                issue_gather(0)
                issue_gather(1)
                emit_pead(0)
                emit_alpha(0)

                prev = None
                q = 0
                for t in range(NT):
                    ps_o = psacc.tile([128, HC], F32, tag="ph")
                    ps_s = pss.tile([128, 8], F32, tag="pss")
                    for k in range(K_t[t]):
                        s, j = divmod(q, SCC)
                        if j == 0:
                            issue_gather(s + 2)
                            emit_pead(s + 1)
                        if j == 2:
                            emit_alpha(s + 1)
                        gA, oz, pead, (pf, pb) = gstate[s]
                        gp = work.tile([128, HC], BF16, tag="gp", bufs=4)
                        for h in range(H):
                            dst_sl = gp[:, h * C:(h + 1) * C]
                            src_sl = gA[:, j, h * C:(h + 1) * C]
                            if h < H // 2:
                                nc.vector.tensor_scalar_mul(
                                    dst_sl, src_sl, pf[:, j, h:h + 1])
                            else:
                                nc.scalar.activation(
                                    dst_sl, src_sl, AF.Copy,
                                    scale=pf[:, j, h:h + 1])
                        first, last = (k == 0), (k == K_t[t] - 1)
                        oh_j = oz[:, j, 0:128]
                        nc.tensor.matmul(ps_o[:], oh_j, gp[:],
                                         start=first, stop=last)
                        nc.tensor.matmul(ps_s[:, 0:4], oh_j, pb[:, j, :],
                                         start=first, stop=last)
                        q += 1
                    ar = fin_pre(t, ps_o, ps_s)
                    if prev is not None:
                        fin_post(*prev)
                    prev = (t, ar)
                fin_post(*prev)

            # ---- tile finalize: softmax-normalize + relu (fused on scalar) --
            def norm_relu(ps_o, ps_s, bias_s, out_dt):
                sp = work.tile([128, 4], F32, tag="sp")
                nc.vector.tensor_scalar_add(sp[:], ps_s[:, 0:4], 1e-16)
                rc = work.tile([128, 4], F32, tag="rc")
                nc.vector.reciprocal(rc[:], sp[:])
                if bias_s is None:
                    ar = work.tile([128, HC], out_dt,
                                   tag=f"ar{out_dt}", bufs=3)
                    for h in range(H):
                        nc.scalar.activation(
                            ar[:, h * C:(h + 1) * C], ps_o[:, h * C:(h + 1) * C],
                            AF.Relu, scale=rc[:, h:h + 1])
                    return ar
                ao = work.tile([128, HC], F32, tag="ao", bufs=2)
                for h in range(H):
                    nc.vector.tensor_scalar_mul(
                        ao[:, h * C:(h + 1) * C], ps_o[:, h * C:(h + 1) * C],
                        rc[:, h:h + 1])
                ab = work.tile([128, HC], F32, tag="ao", bufs=2)
                nc.vector.tensor_add(ab[:], ao[:], bias_s[:])
                ar = work.tile([128, HC], out_dt, tag=f"ar{out_dt}", bufs=3)
                nc.scalar.activation(ar[:], ab[:], AF.Relu)
                return ar

            # ---- layer-0 finalize: transpose + layer-1 linear + group AG ----
            def fin_pre0(t, ps_o, ps_s):
                return norm_relu(ps_o, ps_s, b0_s, BF16)

            def fin_post0(t, ar):
                a0k = []
                for kk in range(KB):
                    pt = pstr.tile([128, 128], BF16, tag="pt", name="ptb")
                    nc.tensor.transpose(pt[:], ar[:, kk * 128:(kk + 1) * 128],
                                        idb_s[:])
                    ak = tp.tile([128, 128], BF16, tag="a1T", name=f"a0k{kk}")
                    nc.vector.tensor_copy(ak[:], pt[:])
                    a0k.append(ak)
                ph1 = psacc.tile([128, HC], F32, tag="ph")
                pa1 = pss.tile([128, 8], F32, tag="pss")
                for kk in range(KB):
                    first, last = (kk == 0), (kk == KB - 1)
                    nc.tensor.matmul(ph1[:], a0k[kk][:], r1h_s[kk][:],
                                     start=first, stop=last)
                    nc.tensor.matmul(pa1[:], a0k[kk][:], r1a_s[kk][:],
                                     start=first, stop=last)
                st = stage.tile([128, TW], BF16, tag="hrow", bufs=4)
                if t % 2 == 0:
                    nc.vector.tensor_copy(st[:, 0:HC], ph1[:])
                else:
                    nc.scalar.activation(st[:, 0:HC], ph1[:], AF.Copy)
                nc.vector.tensor_copy(st[:, HC:HC + 16].bitcast(F32), pa1[:])
                nc.vector.tensor_copy(adts1[:, t, :], pa1[:, 4:8])
                g = t // GPG
                loc = t % GPG
                nc.sync.dma_start(out=H1g[g][loc * 128:(loc + 1) * 128, :],
                                  in_=st[:])
                if loc == GPG - 1:
                    nc.gpsimd.collective_compute(
                        "AllGather", OP.bypass,
                        replica_groups=[list(range(NCORES))],
                        ins=[H1g[g].opt()],
                        outs=[H1[g * NCORES * GR:(g + 1) * NCORES * GR, :].opt()],
                    )

            nc._state.push_named_scope("phaseB")
            agg_layer(H0, ae0_s, adts0, fin_pre0, fin_post0)
            nc._state.pop_named_scope("phaseB")

            # ---- layer-1 aggregation + MLP head per dst tile ----
            def fin_pre1(t, ps_o, ps_s):
                return norm_relu(ps_o, ps_s, b1_s, F32)

            def fin_post1(t, ar):
                h2p = psacc.tile([128, FTS], F32, tag="ph")
                for kk in range(KB):
                    pt = pstr.tile([128, 128], F32, tag="pt", name="ptf")
                    nc.tensor.transpose(pt[:], ar[:, kk * 128:(kk + 1) * 128],
                                        id_s[:])
                    a1k = tp.tile([128, 128], F32, tag="a1T")
                    nc.vector.tensor_copy(a1k[:], pt[:])
                    nc.tensor.matmul(h2p[:], a1k[:], r2_s[kk][:],
                                     start=(kk == 0), stop=(kk == KB - 1))
                if use_l0b:
                    h2b = work.tile([128, FTS], F32, tag="h2b")
                    nc.vector.tensor_add(h2b[:], h2p[:], l0b_s[:])
                else:
                    h2b = h2p
                h2r = work.tile([128, FTS], F32, tag="h2r")
                nc.scalar.activation(h2r[:], h2b[:], AF.Relu)
                pt2 = pstr.tile([128, 128], F32, tag="pt")
                nc.tensor.transpose(pt2[:], h2r[:], id_s[:])
                h2T = tp.tile([128, 128], F32, tag="a1T")
                nc.vector.tensor_copy(h2T[:], pt2[:])
                po = pss.tile([128, 8], F32, tag="pss")
                nc.tensor.matmul(po[:, 0:1], h2T[:], r3_s[:],
                                 start=True, stop=True)
                ob = work.tile([128, 1], F32, tag="ob")
                if use_l1b:
                    nc.vector.tensor_add(ob[:], po[:, 0:1], l1b_s[:])
                else:
                    nc.vector.tensor_copy(ob[:], po[:, 0:1])
                nc.sync.dma_start(out=outp[t * 128:(t + 1) * 128, :], in_=ob[:])

            nc._state.push_named_scope("phaseD")
            ae1_s = cload(ae1[:, :, :], [128, NCHUNK, 4], tag="ae")
            agg_layer(H1, ae1_s, adts1, fin_pre1, fin_post1)
            nc._state.pop_named_scope("phaseD")

    nc.finalize()
    return nc


def _wrap_idx(v, E_pad):
    blk = np.zeros((16, E_pad // 16), np.int16)
    ar = np.arange(E_pad)
    blk[ar % 16, ar // 16] = v.astype(np.int16)
    return np.tile(blk, (8, 1))


def kernel(x, edge_index, edge_weights,
           W0, as0, ad0, We0, ae0, b0,
           W1, as1, ad1, We1, ae1, b1,
           L0W, L0b, L1W, L1b):
    x = np.asarray(x, np.float32)
    N, F_IN = x.shape
    HC = W0.shape[0]
    H, C = np.asarray(as0).shape
    FTS = np.asarray(L0W).shape[0]

    NT = -(-N // (128 * NCORES))
    SHARD = NT * 128
    NP = SHARD * NCORES
    NAG = 1
    for cand in (5, 4, 2, 10):
        if NT % cand == 0:
            NAG = cand
            break
    GPG = NT // NAG
    GR = GPG * 128

    # table-row permutation (group-major) so group AllGathers land contiguous
    nodes = np.arange(NP)
    core = nodes // SHARD
    rr = nodes % SHARD
    gg = rr // GR
    off = rr % GR
    t_of_n = gg * (NCORES * GR) + core * GR + off     # node -> table row

    # ---- edges ----
    ew_in = np.asarray(edge_weights, np.float32)
    src = np.concatenate([np.asarray(edge_index[0]), np.arange(N)])
    dst = np.concatenate([np.asarray(edge_index[1]), np.arange(N)])
    ew = np.concatenate([ew_in, np.full(N, ew_in.mean(), np.float32)])
    order = np.argsort(dst, kind="stable")
    src_s, dst_s, ew_s = src[order], dst[order], ew[order]

    NTG = NP // 128
    tile_of = (dst_s // 128).astype(np.int64)
    tcounts = np.bincount(tile_of, minlength=NTG)
    tstart = np.concatenate([[0], np.cumsum(tcounts)])

    K_t = [max(1, int(max(-(-tcounts[i * NT + t] // 128) for i in range(NCORES))))
           for t in range(NT)]
    NCHUNK = int(sum(K_t))
    E_pad = NCHUNK * 128

    # ---- weight folding (host, O(weights)) ----
    as0 = np.asarray(as0, np.float32)
    ad0 = np.asarray(ad0, np.float32)
    ae0w = np.asarray(ae0, np.float32)
    as1 = np.asarray(as1, np.float32)
    ad1 = np.asarray(ad1, np.float32)
    ae1w = np.asarray(ae1, np.float32)
    W0 = np.asarray(W0, np.float32)
    W1 = np.asarray(W1, np.float32)
    We0 = np.asarray(We0, np.float32)
    We1 = np.asarray(We1, np.float32)

    k0 = (We0.reshape(H, C) * ae0w).sum(1).astype(np.float32)
    k1 = (We1.reshape(H, C) * ae1w).sum(1).astype(np.float32)

    def fold(W, a):
        blk = np.zeros((HC, H), np.float32)
        for h in range(H):
            blk[h * C:(h + 1) * C, h] = a[h]
        return (W.T @ blk).astype(np.float32)

    bf = ml_dtypes.bfloat16
    r0h = W0.T.astype(bf)
    r0a = np.concatenate([fold(W0, as0), fold(W0, ad0)], 1).astype(bf)
    r1h = W1.T.astype(bf)
    r1a = np.concatenate([fold(W1, as1), fold(W1, ad1)], 1).astype(bf)
    r2 = np.asarray(L0W, np.float32).T.copy()
    r3 = np.asarray(L1W, np.float32).T.copy()

    # xT in TABLE-ROW order: column r of xT = x[node(r)]
    inv = np.empty(NP, np.int64)
    inv[t_of_n] = nodes                              # table row -> node
    xa = np.zeros((NP, F_IN), np.float32)
    xa[:N] = x
    xT = np.ascontiguousarray(xa[inv].T).astype(bf)

    ident = np.eye(128, dtype=np.float32)
    identb = np.eye(128, dtype=np.float32).astype(bf)

    use_b0 = bool(np.any(b0))
    use_b1 = bool(np.any(b1))
    use_l0b = bool(np.any(np.asarray(L0b)))
    use_l1b = bool(np.any(np.asarray(L1b)))

    in_maps = []
    for i in range(NCORES):
        srcp = np.zeros(E_pad, np.int64)
        dlocp = np.full(E_pad, -1, np.int64)
        ewp = np.zeros(E_pad, np.float32)
        offq = 0
        for t in range(NT):
            g = i * NT + t
            cnt = int(tcounts[g])
            sl = slice(tstart[g], tstart[g] + cnt)
            srcp[offq:offq + cnt] = t_of_n[src_s[sl]]
            dlocp[offq:offq + cnt] = dst_s[sl] - g * 128
            ewp[offq:offq + cnt] = ew_s[sl]
            offq += K_t[t] * 128
        ae0p = (ewp[:, None] * k0[None, :]).reshape(NCHUNK, 128, 4).transpose(1, 0, 2)
        ae1p = (ewp[:, None] * k1[None, :]).reshape(NCHUNK, 128, 4).transpose(1, 0, 2)
        # one-hot blocks: ohz[e, q, d] = ohb, ohz[d, q, 128+e] = oht
        ohcube = np.zeros((NCHUNK, 128, 128), np.float32)  # [q, e, d]
        dl2 = dlocp.reshape(NCHUNK, 128)
        valid = dl2 >= 0
        qs, es = np.nonzero(valid)
        ohcube[qs, es, dl2[qs, es]] = 1.0
        ohz_np = np.empty((128, NCHUNK, 256), bf)
        ohz_np[:, :, 0:128] = ohcube.transpose(1, 0, 2).astype(bf)
        ohz_np[:, :, 128:256] = ohcube.transpose(2, 0, 1).astype(bf)
        # own dst-tile table rows for the layer-0 alpha_dst gather
        trows = np.empty((NT, 128), np.int64)
        for t in range(NT):
            base = t_of_n[i * SHARD + t * 128]
            trows[t] = base + np.arange(128)
        trw_np = _wrap_idx(trows.reshape(-1), NT * 128)
        im = {
            "xT": xT, "r0h": r0h, "r0a": r0a, "r1h": r1h, "r1a": r1a,
            "r2": r2, "r3": r3, "ident": ident, "identb": identb,
            "srcw": _wrap_idx(srcp, E_pad), "trw": trw_np,
            "ohz": ohz_np,
            "ae0": np.ascontiguousarray(ae0p),
            "ae1": np.ascontiguousarray(ae1p),
        }
        if use_b0:
            im["b0t"] = np.tile(np.asarray(b0, np.float32)[None, :], (128, 1))
        if use_b1:
            im["b1t"] = np.tile(np.asarray(b1, np.float32)[None, :], (128, 1))
        if use_l0b:
            im["l0bt"] = np.tile(np.asarray(L0b, np.float32)[None, :], (128, 1))
        if use_l1b:
            im["l1bt"] = np.tile(np.asarray(L1b, np.float32).reshape(1, 1), (128, 1))
        in_maps.append(im)

    nc = _build_program(NP, F_IN, HC, H, C, NT, K_t, FTS, NAG,
                        use_b0, use_b1, use_l0b, use_l1b)
    res = run_bass_kernel_spmd(nc, in_maps, list(range(NCORES)))
    out = np.concatenate([res.results[i]["out"][:, 0] for i in range(NCORES)])
    return out[:N].astype(np.float32)


# revision 21
# speedup vs baseline: 1.4578x; 1.0796x over previous
"""2-layer GAT + MLP head on 8 TRN2 NeuronCores.

Strategy (dst-sharded, software-pipelined):
- Nodes padded to NP=20480; each core owns a contiguous 2560-dst shard.
- Edges (incl. self-loops, PyG mean-fill edge attr) sorted by dst,
  grouped into 128-dst tiles, padded per tile-slot to a chunk count K_t
  shared by all cores (SPMD: one program).
- Per layer a node table [NP, 640] bf16 in HBM: cols [0:512) = h,
  bytes [1024:1056) = asrc|adst (f32 bits). Rows permuted group-major
  so layer-1 tables assemble from per-group AllGathers (Shared HBM).
- Aggregation per 128-edge chunk: gather rows by src (1 DMA per
  8-chunk super), one-hot blocks ([e,d] bf16 + [d,e] bf16) streamed as
  one fused ohz tensor; p = exp(lrelu(asrc+adst+aedge)) with adst
  expanded via one-hot matmul; out[dst] += (p*h) via one-hot matmul in
  PSUM; denominator via second matmul with rhs=p.
- Pipelining: gathers issued 2 supers ahead, alpha chains 1 super
  ahead, tile finalize deferred by 1 tile so the in-order PE stream
  never waits on vector/scalar chains.
- dst-tile alphas for layer 1 captured into SBUF during fin0 (no
  gather); layer-0 ones via one batched 2560-row gather from H0.
"""

import numpy as np
import ml_dtypes

import concourse.bacc as bacc
import concourse.bass as bass
import concourse.mybir as mybir
import concourse.tile as tile
from concourse.bass_utils import run_bass_kernel_spmd

F32 = mybir.dt.float32
BF16 = mybir.dt.bfloat16
I16 = mybir.dt.int16
AF = mybir.ActivationFunctionType
OP = mybir.AluOpType

NCORES = 8
SCC = 8  # chunks (of 128 edges) per gather super-chunk


def _bcast4(ap_tile, j, reps):
    """[128, SCC, 4] tile -> [128, 4, reps] zero-step broadcast AP of slot j."""
    sl = ap_tile[:, j, :]
    return bass.AP(sl.tensor, sl.offset, [list(sl.ap[0]), list(sl.ap[-1]), [0, reps]])


def _build_program(NP, F_IN, HC, H, C, NT, K_t, FTS, GS, use_b0, use_b1,
                   use_l0b, use_l1b):
    NCHUNK = int(sum(K_t))
    E_pad = NCHUNK * 128
    SW = E_pad // 16
    TW = HC + 128  # bf16 table row: h | asrc,adst (f32 bits) | pad
    KB = HC // 128
    NAG = len(GS)            # allgather groups (variable tile counts)
    gstart_t = [0]
    for s_ in GS:
        gstart_t.append(gstart_t[-1] + s_)
    grp_of_t = []
    for g, s_ in enumerate(GS):
        grp_of_t += [g] * s_
    NSUP = -(-NCHUNK // SCC)
    MT = NP // 128

    # chunk q -> owning dst tile
    t_of_q = []
    for t in range(NT):
        t_of_q += [t] * K_t[t]

    nc = bacc.Bacc(dynamic_dma_scratch_size=65536, num_swdge_queues=4)
    P = nc.declare_dram_parameter

    xT = P("xT", [F_IN, NP], BF16, isOutput=False)
    r0h = P("r0h", [F_IN, HC], BF16, isOutput=False)
    r0a = P("r0a", [F_IN, 8], BF16, isOutput=False)
    r1h = P("r1h", [HC, HC], BF16, isOutput=False)
    r1a = P("r1a", [HC, 8], BF16, isOutput=False)
    r2 = P("r2", [HC, FTS], F32, isOutput=False)
    r3 = P("r3", [FTS, 1], F32, isOutput=False)
    ident = P("ident", [128, 128], F32, isOutput=False)
    identb = P("identb", [128, 128], BF16, isOutput=False)
    srcw = P("srcw", [128, SW], I16, isOutput=False)
    trw = P("trw", [128, NT * 8], I16, isOutput=False)
    ohz = P("ohz", [128, NCHUNK, 256], BF16, isOutput=False)
    ae0 = P("ae0", [128, NCHUNK, 4], F32, isOutput=False)
    ae1 = P("ae1", [128, NCHUNK, 4], F32, isOutput=False)
    if use_b0:
        b0t = P("b0t", [128, HC], F32, isOutput=False)
    if use_b1:
        b1t = P("b1t", [128, HC], F32, isOutput=False)
    if use_l0b:
        l0bt = P("l0bt", [128, FTS], F32, isOutput=False)
    if use_l1b:
        l1bt = P("l1bt", [128, 1], F32, isOutput=False)
    outp = P("out", [NT * 128, 1], F32, isOutput=True)

    with tile.TileContext(nc) as tc:
        with (
            tc.tile_pool(name="const", bufs=1) as const,
            tc.tile_pool(name="stage", bufs=2) as stage,
            tc.tile_pool(name="work", bufs=3) as work,
            tc.tile_pool(name="tp", bufs=6) as tp,
            tc.tile_pool(name="adp", bufs=1) as adp,
            tc.tile_pool(name="psacc", bufs=3, space="PSUM") as psacc,
            tc.tile_pool(name="pss", bufs=2, space="PSUM") as pss,
            tc.tile_pool(name="pstr", bufs=3, space="PSUM") as pstr,
            tc.tile_pool(name="dram", bufs=1, space="DRAM") as dram,
        ):
            H0 = dram.tile([NP, TW], BF16, tag="H0")
            H1 = dram.tile([NP, TW], BF16, tag="H1")
            H1g = [dram.tile([GS[g] * 128, TW], BF16, tag=f"H1g{g}",
                             name=f"H1g{g}") for g in range(NAG)]

            _cn = [0]

            def cload(ap_in, shape, dt=F32, tag=None):
                _cn[0] += 1
                cname = tag or f"c{_cn[0]}"
                t = const.tile(shape, dt, tag=cname, name=f"{cname}_{_cn[0]}")
                nc.sync.dma_start(out=t[:], in_=ap_in)
                return t

            r0h_s = cload(r0h[:, :], [F_IN, HC], BF16)
            r0a_s = cload(r0a[:, :], [F_IN, 8], BF16)
            r1h_s = [cload(r1h[k * 128:(k + 1) * 128, :], [128, HC], BF16)
                     for k in range(KB)]
            r1a_s = [cload(r1a[k * 128:(k + 1) * 128, :], [128, 8], BF16)
                     for k in range(KB)]
            r2_s = [cload(r2[k * 128:(k + 1) * 128, :], [128, FTS]) for k in range(KB)]
            r3_s = cload(r3[:, :], [FTS, 1])
            id_s = cload(ident[:, :], [128, 128])
            idb_s = cload(identb[:, :], [128, 128], BF16)
            srcw_s = cload(srcw[:, :], [128, SW], I16)
            trw_s = cload(trw[:, :], [128, NT * 8], I16)
            ae0_s = cload(ae0[:, :, :], [128, NCHUNK, 4], tag="ae")
            b0_s = cload(b0t[:, :], [128, HC]) if use_b0 else None
            b1_s = cload(b1t[:, :], [128, HC]) if use_b1 else None
            l0b_s = cload(l0bt[:, :], [128, FTS]) if use_l0b else None
            l1b_s = cload(l1bt[:, :], [128, 1]) if use_l1b else None
            xTs = cload(xT[:, :], [F_IN, NP], BF16, tag="xT")

            # persistent alpha_dst for layer-1's own dst tiles (filled in fin0)
            adts1 = adp.tile([128, NT, 4], BF16, tag="adts1")

            # ---- phase A: layer-0 table (xT pre-permuted to row order) ----
            # PSUM alternates between psacc and pstr pools so the PE streams
            # continuously; stores batched in pairs across sync+scalar queues.
            nc._state.push_named_scope("phaseA")
            st2 = None
            for mr in range(MT):
                lx = xTs[:, mr * 128:(mr + 1) * 128]
                pool = psacc if mr % 2 == 0 else pstr
                ph = pool.tile([128, HC], F32, tag="ph" if mr % 2 == 0 else "pt",
                               name="phA")
                nc.tensor.matmul(ph[:], lx, r0h_s[:], start=True, stop=True)
                pa = pss.tile([128, 8], F32, tag="pss")
                nc.tensor.matmul(pa[:], lx, r0a_s[:], start=True, stop=True)
                half = mr % 2
                if half == 0:
                    st2 = stage.tile([128, 2, TW], BF16, tag="hrow2", bufs=4)
                if mr % 2 == 0:
                    nc.vector.tensor_copy(st2[:, half, 0:HC], ph[:])
                    nc.vector.tensor_copy(
                        st2[:, half, HC:HC + 16].bitcast(F32), pa[:])
                else:
                    nc.scalar.activation(st2[:, half, 0:HC], ph[:], AF.Copy)
                    nc.vector.tensor_copy(
                        st2[:, half, HC:HC + 16].bitcast(F32), pa[:])
                if half == 1:
                    dst = H0[(mr - 1) * 128:(mr + 1) * 128, :].rearrange(
                        "(j p) c -> p j c", p=128)
                    eng = nc.sync if (mr // 2) % 2 == 0 else nc.scalar
                    eng.dma_start(out=dst, in_=st2[:])
            nc._state.pop_named_scope("phaseA")

            # ---- layer-0 alpha_dst for own dst tiles: one batched gather ----
            adt_all = adp.tile([128, NT, 128], BF16, tag="adta")
            nc.gpsimd.dma_gather(
                adt_all[:], H0[:, HC:TW], trw_s[:, 0:NT * 8],
                NT * 128, NT * 128, 128, elem_step=TW,
                single_packet=False, queue_num=3)
            adts0 = adp.tile([128, NT, 4], BF16, tag="adts0")
            nc.vector.tensor_copy(adts0[:], adt_all[:, :, 8:16].bitcast(F32))

            # ---- aggregation over one layer's edges ----
            def agg_layer(tbl, ae_s, adts, fin_pre, fin_post):
                gstate = {}

                def nch_of(s):
                    return min(SCC, NCHUNK - s * SCC)

                def issue_gather(s):
                    if s >= NSUP:
                        return
                    nch = nch_of(s)
                    gA = stage.tile([128, SCC, TW], BF16, tag="gA",
                                    name="gA", bufs=3)
                    c0 = s * SCC * 8
                    base = 0
                    nq = min(4, nch)
                    for qi in range(nq):
                        take = (nch - base + (nq - qi) - 1) // (nq - qi)
                        nc.gpsimd.dma_gather(
                            gA[:, base:base + take, :], tbl[:, :],
                            srcw_s[:, c0 + base * 8:c0 + (base + take) * 8],
                            take * 128, take * 128, TW,
                            single_packet=False, queue_num=qi)
                        base += take
                    oz = stage.tile([128, SCC, 256], BF16, tag="ohz",
                                    name="oz", bufs=4)
                    nc.sync.dma_start(
                        out=oz[:, 0:nch, :],
                        in_=ohz[:, s * SCC:s * SCC + nch, :])
                    gstate[s] = [gA, oz, None, None]

                def emit_pead(s):
                    if s >= NSUP:
                        return
                    nch = nch_of(s)
                    oz = gstate[s][1]
                    pead = pstr.tile([128, SCC * 4], F32, tag="pt", name="pead")
                    for jj in range(nch):
                        q = s * SCC + jj
                        nc.tensor.matmul(
                            pead[:, jj * 4:(jj + 1) * 4],
                            oz[:, jj, 128:256], adts[:, t_of_q[q], :],
                            start=True, stop=True)
                    gstate[s][2] = pead

                def emit_alpha(s):
                    if s >= NSUP:
                        return
                    nch = nch_of(s)
                    gA, oz, pead, _ = gstate[s]
                    asrc = gA[:, 0:nch, HC:HC + 8].bitcast(F32)
                    t0 = work.tile([128, SCC, 4], F32, tag="t0", bufs=2)
                    nc.vector.tensor_add(
                        t0[:, 0:nch, :], asrc,
                        ae_s[:, s * SCC:s * SCC + nch, :])
                    t1 = work.tile([128, SCC, 4], F32, tag="t1", bufs=2)
                    nc.vector.tensor_add(
                        t1[:, 0:nch, :], t0[:, 0:nch, :],
                        pead[:, 0:nch * 4].rearrange("x (a b) -> x a b", b=4))
                    t2 = work.tile([128, SCC, 4], F32, tag="t2", bufs=2)
                    nc.scalar.activation(
                        t2[:, 0:nch, :], t1[:, 0:nch, :], AF.Copy, scale=0.2)
                    tl = work.tile([128, SCC, 4], F32, tag="tl", bufs=2)
                    nc.vector.tensor_max(
                        tl[:, 0:nch, :], t1[:, 0:nch, :], t2[:, 0:nch, :])
                    pf = work.tile([128, SCC, 4], F32, tag="pf", bufs=3)
                    nc.scalar.activation(pf[:, 0:nch, :], tl[:, 0:nch, :], AF.Exp)
                    pb = work.tile([128, SCC, 4], BF16, tag="p", bufs=3)
                    nc.scalar.activation(pb[:, 0:nch, :], pf[:, 0:nch, :], AF.Copy)
                    gstate[s][3] = (pf, pb)

                issue_gather(0)
                issue_gather(1)
                emit_pead(0)
                emit_alpha(0)

                HV = H - 1  # heads on vector (one 3D op); last head on scalar
                pend = None
                q = 0
                for t in range(NT):
                    ps_o = psacc.tile([128, HC], F32, tag="ph")
                    ps_s = pss.tile([128, 8], F32, tag="pss")
                    fin_t, fin_ar = None, None
                    for k in range(K_t[t]):
                        s, j = divmod(q, SCC)
                        if j == 0:
                            issue_gather(s + 2)
                            emit_pead(s + 1)
                        if j == 2:
                            emit_alpha(s + 1)
                        if pend is not None and k == min(2, K_t[t] - 1):
                            fin_t = pend[0]
                            fin_ar = fin_pre(*pend)
                            pend = None
                        gA, oz, pead, (pf, pb) = gstate[s]
                        gp = work.tile([128, HC], BF16, tag="gp", bufs=4)
                        nc.vector.tensor_mul(
                            gp[:, 0:HV * C].rearrange("x (h c) -> x h c", h=HV),
                            gA[:, j, 0:HV * C].rearrange("x (h c) -> x h c", h=HV),
                            _bcast4(pf[:, :, 0:HV], j, C))
                        nc.scalar.activation(
                            gp[:, HV * C:H * C], gA[:, j, HV * C:H * C],
                            AF.Copy, scale=pf[:, j, HV:HV + 1])
                        first, last = (k == 0), (k == K_t[t] - 1)
                        oh_j = oz[:, j, 0:128]
                        nc.tensor.matmul(ps_o[:], oh_j, gp[:],
                                         start=first, stop=last)
                        nc.tensor.matmul(ps_s[:, 0:4], oh_j, pb[:, j, :],
                                         start=first, stop=last)
                        q += 1
                    if fin_ar is not None:
                        fin_post(fin_t, fin_ar)
                    pend = (t, ps_o, ps_s)
                ar = fin_pre(*pend)
                fin_post(pend[0], ar)

            # ---- tile finalize: softmax-normalize + relu (fused on scalar) --
            def norm_relu(ps_o, ps_s, bias_s, out_dt):
                sp = work.tile([128, 4], F32, tag="sp")
                nc.vector.tensor_scalar_add(sp[:], ps_s[:, 0:4], 1e-16)
                rc = work.tile([128, 4], F32, tag="rc")
                nc.vector.reciprocal(rc[:], sp[:])
                if bias_s is None:
                    ar = work.tile([128, HC], out_dt,
                                   tag=f"ar{out_dt}", bufs=3)
                    for h in range(H):
                        nc.scalar.activation(
                            ar[:, h * C:(h + 1) * C], ps_o[:, h * C:(h + 1) * C],
                            AF.Relu, scale=rc[:, h:h + 1])
                    return ar
                ao = work.tile([128, HC], F32, tag="ao", bufs=2)
                for h in range(H):
                    nc.vector.tensor_scalar_mul(
                        ao[:, h * C:(h + 1) * C], ps_o[:, h * C:(h + 1) * C],
                        rc[:, h:h + 1])
                ab = work.tile([128, HC], F32, tag="ao", bufs=2)
                nc.vector.tensor_add(ab[:], ao[:], bias_s[:])
                ar = work.tile([128, HC], out_dt, tag=f"ar{out_dt}", bufs=3)
                nc.scalar.activation(ar[:], ab[:], AF.Relu)
                return ar

            # ---- layer-0 finalize: transpose + layer-1 linear + group AG ----
            def fin_pre0(t, ps_o, ps_s):
                return norm_relu(ps_o, ps_s, b0_s, BF16)

            def fin_post0(t, ar):
                a0k = []
                for kk in range(KB):
                    pt = pstr.tile([128, 128], BF16, tag="pt", name="ptb")
                    nc.tensor.transpose(pt[:], ar[:, kk * 128:(kk + 1) * 128],
                                        idb_s[:])
                    ak = tp.tile([128, 128], BF16, tag="a1T", name=f"a0k{kk}")
                    nc.vector.tensor_copy(ak[:], pt[:])
                    a0k.append(ak)
                ph1 = psacc.tile([128, HC], F32, tag="ph")
                pa1 = pss.tile([128, 8], F32, tag="pss")
                for kk in range(KB):
                    first, last = (kk == 0), (kk == KB - 1)
                    nc.tensor.matmul(ph1[:], a0k[kk][:], r1h_s[kk][:],
                                     start=first, stop=last)
                    nc.tensor.matmul(pa1[:], a0k[kk][:], r1a_s[kk][:],
                                     start=first, stop=last)
                st = stage.tile([128, TW], BF16, tag="hrow", bufs=4)
                if t % 2 == 0:
                    nc.vector.tensor_copy(st[:, 0:HC], ph1[:])
                else:
                    nc.scalar.activation(st[:, 0:HC], ph1[:], AF.Copy)
                nc.vector.tensor_copy(st[:, HC:HC + 16].bitcast(F32), pa1[:])
                nc.vector.tensor_copy(adts1[:, t, :], pa1[:, 4:8])
                g = grp_of_t[t]
                loc = t - gstart_t[g]
                nc.sync.dma_start(out=H1g[g][loc * 128:(loc + 1) * 128, :],
                                  in_=st[:])
                if loc == GS[g] - 1:
                    r0_, r1_ = (NCORES * 128 * gstart_t[g],
                                NCORES * 128 * gstart_t[g + 1])
                    nc.gpsimd.collective_compute(
                        "AllGather", OP.bypass,
                        replica_groups=[list(range(NCORES))],
                        ins=[H1g[g].opt()],
                        outs=[H1[r0_:r1_, :].opt()],
                    )

            nc._state.push_named_scope("phaseB")
            agg_layer(H0, ae0_s, adts0, fin_pre0, fin_post0)
            nc._state.pop_named_scope("phaseB")

            # ---- layer-1 aggregation + MLP head per dst tile ----
            def fin_pre1(t, ps_o, ps_s):
                return norm_relu(ps_o, ps_s, b1_s, F32)

            def fin_post1(t, ar):
                h2p = psacc.tile([128, FTS], F32, tag="ph")
                for kk in range(KB):
                    pt = pstr.tile([128, 128], F32, tag="pt", name="ptf")
                    nc.tensor.transpose(pt[:], ar[:, kk * 128:(kk + 1) * 128],
                                        id_s[:])
                    a1k = tp.tile([128, 128], F32, tag="a1T")
                    nc.vector.tensor_copy(a1k[:], pt[:])
                    nc.tensor.matmul(h2p[:], a1k[:], r2_s[kk][:],
                                     start=(kk == 0), stop=(kk == KB - 1))
                if use_l0b:
                    h2b = work.tile([128, FTS], F32, tag="h2b")
                    nc.vector.tensor_add(h2b[:], h2p[:], l0b_s[:])
                else:
                    h2b = h2p
                h2r = work.tile([128, FTS], F32, tag="h2r")
                nc.scalar.activation(h2r[:], h2b[:], AF.Relu)
                pt2 = pstr.tile([128, 128], F32, tag="pt")
                nc.tensor.transpose(pt2[:], h2r[:], id_s[:])
                h2T = tp.tile([128, 128], F32, tag="a1T")
                nc.vector.tensor_copy(h2T[:], pt2[:])
                po = pss.tile([128, 8], F32, tag="pss")
                nc.tensor.matmul(po[:, 0:1], h2T[:], r3_s[:],
                                 start=True, stop=True)
                ob = work.tile([128, 1], F32, tag="ob")
                if use_l1b:
                    nc.vector.tensor_add(ob[:], po[:, 0:1], l1b_s[:])
                else:
                    nc.vector.tensor_copy(ob[:], po[:, 0:1])
                nc.sync.dma_start(out=outp[t * 128:(t + 1) * 128, :], in_=ob[:])

            nc._state.push_named_scope("phaseD")
            ae1_s = cload(ae1[:, :, :], [128, NCHUNK, 4], tag="ae")
            agg_layer(H1, ae1_s, adts1, fin_pre1, fin_post1)
            nc._state.pop_named_scope("phaseD")

    nc.finalize()
    return nc


def _wrap_idx(v, E_pad):
    blk = np.zeros((16, E_pad // 16), np.int16)
    ar = np.arange(E_pad)
    blk[ar % 16, ar // 16] = v.astype(np.int16)
    return np.tile(blk, (8, 1))


def kernel(x, edge_index, edge_weights,
           W0, as0, ad0, We0, ae0, b0,
           W1, as1, ad1, We1, ae1, b1,
           L0W, L0b, L1W, L1b):
    x = np.asarray(x, np.float32)
    N, F_IN = x.shape
    HC = W0.shape[0]
    H, C = np.asarray(as0).shape
    FTS = np.asarray(L0W).shape[0]

    NT = -(-N // (128 * NCORES))
    SHARD = NT * 128
    NP = SHARD * NCORES

    # AllGather group sizes (in dst tiles): big groups early (lots of overlap
    # time), shrinking tail so the last collective is small.
    if NT == 20:
        GS = [6, 5, 4, 2, 2, 1]
    else:
        GS = []
        rem = NT
        while rem > 0:
            s = max(1, -(-rem // 3))
            GS.append(min(s, rem))
            rem -= GS[-1]
    gstart_t = np.concatenate([[0], np.cumsum(GS)]).astype(np.int64)
    grp_of_t = np.zeros(NT, np.int64)
    for g, s in enumerate(GS):
        grp_of_t[gstart_t[g]:gstart_t[g + 1]] = g
    GS_arr = np.asarray(GS, np.int64)

    # table-row permutation (group-major) so group AllGathers land contiguous
    nodes = np.arange(NP)
    core = nodes // SHARD
    rr = nodes % SHARD
    tt = rr // 128
    gg = grp_of_t[tt]
    off = (tt - gstart_t[gg]) * 128 + rr % 128
    t_of_n = (NCORES * 128 * gstart_t[gg] + core * (GS_arr[gg] * 128)
              + off)                                   # node -> table row

    # ---- edges ----
    ew_in = np.asarray(edge_weights, np.float32)
    src = np.concatenate([np.asarray(edge_index[0]), np.arange(N)])
    dst = np.concatenate([np.asarray(edge_index[1]), np.arange(N)])
    ew = np.concatenate([ew_in, np.full(N, ew_in.mean(), np.float32)])
    order = np.argsort(dst, kind="stable")
    src_s, dst_s, ew_s = src[order], dst[order], ew[order]

    NTG = NP // 128
    tile_of = (dst_s // 128).astype(np.int64)
    tcounts = np.bincount(tile_of, minlength=NTG)
    tstart = np.concatenate([[0], np.cumsum(tcounts)])

    K_t = [max(1, int(max(-(-tcounts[i * NT + t] // 128) for i in range(NCORES))))
           for t in range(NT)]
    NCHUNK = int(sum(K_t))
    E_pad = NCHUNK * 128

    # ---- weight folding (host, O(weights)) ----
    as0 = np.asarray(as0, np.float32)
    ad0 = np.asarray(ad0, np.float32)
    ae0w = np.asarray(ae0, np.float32)
    as1 = np.asarray(as1, np.float32)
    ad1 = np.asarray(ad1, np.float32)
    ae1w = np.asarray(ae1, np.float32)
    W0 = np.asarray(W0, np.float32)
    W1 = np.asarray(W1, np.float32)
    We0 = np.asarray(We0, np.float32)
    We1 = np.asarray(We1, np.float32)

    k0 = (We0.reshape(H, C) * ae0w).sum(1).astype(np.float32)
    k1 = (We1.reshape(H, C) * ae1w).sum(1).astype(np.float32)

    def fold(W, a):
        blk = np.zeros((HC, H), np.float32)
        for h in range(H):
            blk[h * C:(h + 1) * C, h] = a[h]
        return (W.T @ blk).astype(np.float32)

    bf = ml_dtypes.bfloat16
    r0h = W0.T.astype(bf)
    r0a = np.concatenate([fold(W0, as0), fold(W0, ad0)], 1).astype(bf)
    r1h = W1.T.astype(bf)
    r1a = np.concatenate([fold(W1, as1), fold(W1, ad1)], 1).astype(bf)
    r2 = np.asarray(L0W, np.float32).T.copy()
    r3 = np.asarray(L1W, np.float32).T.copy()

    # xT in TABLE-ROW order: column r of xT = x[node(r)]
    inv = np.empty(NP, np.int64)
    inv[t_of_n] = nodes                              # table row -> node
    xa = np.zeros((NP, F_IN), np.float32)
    xa[:N] = x
    xT = np.ascontiguousarray(xa[inv].T).astype(bf)

    ident = np.eye(128, dtype=np.float32)
    identb = np.eye(128, dtype=np.float32).astype(bf)

    use_b0 = bool(np.any(b0))
    use_b1 = bool(np.any(b1))
    use_l0b = bool(np.any(np.asarray(L0b)))
    use_l1b = bool(np.any(np.asarray(L1b)))

    in_maps = []
    for i in range(NCORES):
        srcp = np.zeros(E_pad, np.int64)
        dlocp = np.full(E_pad, -1, np.int64)
        ewp = np.zeros(E_pad, np.float32)
        offq = 0
        for t in range(NT):
            g = i * NT + t
            cnt = int(tcounts[g])
            sl = slice(tstart[g], tstart[g] + cnt)
            srcp[offq:offq + cnt] = t_of_n[src_s[sl]]
            dlocp[offq:offq + cnt] = dst_s[sl] - g * 128
            ewp[offq:offq + cnt] = ew_s[sl]
            offq += K_t[t] * 128
        ae0p = (ewp[:, None] * k0[None, :]).reshape(NCHUNK, 128, 4).transpose(1, 0, 2)
        ae1p = (ewp[:, None] * k1[None, :]).reshape(NCHUNK, 128, 4).transpose(1, 0, 2)
        # one-hot blocks: ohz[e, q, d] = ohb, ohz[d, q, 128+e] = oht
        ohcube = np.zeros((NCHUNK, 128, 128), np.float32)  # [q, e, d]
        dl2 = dlocp.reshape(NCHUNK, 128)
        valid = dl2 >= 0
        qs, es = np.nonzero(valid)
        ohcube[qs, es, dl2[qs, es]] = 1.0
        ohz_np = np.empty((128, NCHUNK, 256), bf)
        ohz_np[:, :, 0:128] = ohcube.transpose(1, 0, 2).astype(bf)
        ohz_np[:, :, 128:256] = ohcube.transpose(2, 0, 1).astype(bf)
        # own dst-tile table rows for the layer-0 alpha_dst gather
        trows = np.empty((NT, 128), np.int64)
        for t in range(NT):
            base = t_of_n[i * SHARD + t * 128]
            trows[t] = base + np.arange(128)
        trw_np = _wrap_idx(trows.reshape(-1), NT * 128)
        im = {
            "xT": xT, "r0h": r0h, "r0a": r0a, "r1h": r1h, "r1a": r1a,
            "r2": r2, "r3": r3, "ident": ident, "identb": identb,
            "srcw": _wrap_idx(srcp, E_pad), "trw": trw_np,
            "ohz": ohz_np,
            "ae0": np.ascontiguousarray(ae0p),
            "ae1": np.ascontiguousarray(ae1p),
        }
        if use_b0:
            im["b0t"] = np.tile(np.asarray(b0, np.float32)[None, :], (128, 1))
        if use_b1:
            im["b1t"] = np.tile(np.asarray(b1, np.float32)[None, :], (128, 1))
        if use_l0b:
            im["l0bt"] = np.tile(np.asarray(L0b, np.float32)[None, :], (128, 1))
        if use_l1b:
            im["l1bt"] = np.tile(np.asarray(L1b, np.float32).reshape(1, 1), (128, 1))
        in_maps.append(im)

    nc = _build_program(NP, F_IN, HC, H, C, NT, K_t, FTS, GS,
                        use_b0, use_b1, use_l0b, use_l1b)
    res = run_bass_kernel_spmd(nc, in_maps, list(range(NCORES)))
    out = np.concatenate([res.results[i]["out"][:, 0] for i in range(NCORES)])
    return out[:N].astype(np.float32)


# revision 22
# speedup vs baseline: 1.6575x; 1.1370x over previous
"""2-layer GAT + MLP head on 8 TRN2 NeuronCores.

Strategy (dst-sharded, software-pipelined):
- Nodes padded to NP=20480; each core owns a contiguous 2560-dst shard.
- Edges (incl. self-loops, PyG mean-fill edge attr) sorted by dst,
  grouped into 128-dst tiles, padded per tile-slot to a chunk count K_t
  shared by all cores (SPMD: one program).
- Per layer a node table [NP, 640] bf16 in HBM: cols [0:512) = h,
  bytes [1024:1056) = asrc|adst (f32 bits). Rows permuted group-major
  so layer-1 tables assemble from per-group AllGathers (Shared HBM).
- Aggregation per 128-edge chunk: gather rows by src (1 DMA per
  8-chunk super), one-hot blocks ([e,d] bf16 + [d,e] bf16) streamed as
  one fused ohz tensor; p = exp(lrelu(asrc+adst+aedge)) with adst
  expanded via one-hot matmul; out[dst] += (p*h) via one-hot matmul in
  PSUM; denominator via second matmul with rhs=p.
- Pipelining: gathers issued 2 supers ahead, alpha chains 1 super
  ahead, tile finalize deferred by 1 tile so the in-order PE stream
  never waits on vector/scalar chains.
- dst-tile alphas for layer 1 captured into SBUF during fin0 (no
  gather); layer-0 ones via one batched 2560-row gather from H0.
"""

import numpy as np
import ml_dtypes

import concourse.bacc as bacc
import concourse.bass as bass
import concourse.mybir as mybir
import concourse.tile as tile
from concourse.bass_utils import run_bass_kernel_spmd

F32 = mybir.dt.float32
F8 = mybir.dt.float8e4
BF16 = mybir.dt.bfloat16
I16 = mybir.dt.int16
AF = mybir.ActivationFunctionType
OP = mybir.AluOpType

NCORES = 8
SCC = 8  # chunks (of 128 edges) per gather super-chunk


def _bcast4(ap_tile, j, reps):
    """[128, SCC, 4] tile -> [128, 4, reps] zero-step broadcast AP of slot j."""
    sl = ap_tile[:, j, :]
    return bass.AP(sl.tensor, sl.offset, [list(sl.ap[0]), list(sl.ap[-1]), [0, reps]])


def _build_program(NP, F_IN, HC, H, C, NT, K_t, FTS, GS, use_b0, use_b1,
                   use_l0b, use_l1b):
    NCHUNK = int(sum(K_t))
    E_pad = NCHUNK * 128
    SW = E_pad // 16
    TW = HC + 128  # bf16 table row: h | asrc,adst (f32 bits) | pad
    KB = HC // 128
    NAG = len(GS)            # allgather groups (variable tile counts)
    gstart_t = [0]
    for s_ in GS:
        gstart_t.append(gstart_t[-1] + s_)
    grp_of_t = []
    for g, s_ in enumerate(GS):
        grp_of_t += [g] * s_
    NSUP = -(-NCHUNK // SCC)
    MT = NP // 128

    # chunk q -> owning dst tile
    t_of_q = []
    for t in range(NT):
        t_of_q += [t] * K_t[t]

    nc = bacc.Bacc(dynamic_dma_scratch_size=65536, num_swdge_queues=4)
    P = nc.declare_dram_parameter

    xT = P("xT", [F_IN, NP], BF16, isOutput=False)
    r0h = P("r0h", [F_IN, HC], BF16, isOutput=False)
    r0a = P("r0a", [F_IN, 8], BF16, isOutput=False)
    r1h = P("r1h", [HC, HC], BF16, isOutput=False)
    r1a = P("r1a", [HC, 8], BF16, isOutput=False)
    r2 = P("r2", [HC, FTS], F32, isOutput=False)
    r3 = P("r3", [FTS, 1], F32, isOutput=False)
    ident = P("ident", [128, 128], F32, isOutput=False)
    identb = P("identb", [128, 128], BF16, isOutput=False)
    srcw = P("srcw", [128, SW], I16, isOutput=False)
    trw = P("trw", [128, NT * 8], I16, isOutput=False)
    ohz = P("ohz", [128, NCHUNK, 256], F8, isOutput=False)
    ae0 = P("ae0", [128, NCHUNK, 4], F32, isOutput=False)
    ae1 = P("ae1", [128, NCHUNK, 4], F32, isOutput=False)
    if use_b0:
        b0t = P("b0t", [128, HC], F32, isOutput=False)
    if use_b1:
        b1t = P("b1t", [128, HC], F32, isOutput=False)
    if use_l0b:
        l0bt = P("l0bt", [128, FTS], F32, isOutput=False)
    if use_l1b:
        l1bt = P("l1bt", [128, 1], F32, isOutput=False)
    outp = P("out", [NT * 128, 1], F32, isOutput=True)

    with tile.TileContext(nc) as tc:
        with (
            tc.tile_pool(name="const", bufs=1) as const,
            tc.tile_pool(name="stage", bufs=2) as stage,
            tc.tile_pool(name="work", bufs=3) as work,
            tc.tile_pool(name="tp", bufs=6) as tp,
            tc.tile_pool(name="adp", bufs=1) as adp,
            tc.tile_pool(name="psacc", bufs=3, space="PSUM") as psacc,
            tc.tile_pool(name="pss", bufs=2, space="PSUM") as pss,
            tc.tile_pool(name="pstr", bufs=3, space="PSUM") as pstr,
            tc.tile_pool(name="dram", bufs=1, space="DRAM") as dram,
        ):
            H0 = dram.tile([NP, TW], BF16, tag="H0")
            H1 = dram.tile([NP, TW], BF16, tag="H1")
            H1g = [dram.tile([GS[g] * 128, TW], BF16, tag=f"H1g{g}",
                             name=f"H1g{g}") for g in range(NAG)]

            _cn = [0]

            def cload(ap_in, shape, dt=F32, tag=None):
                _cn[0] += 1
                cname = tag or f"c{_cn[0]}"
                t = const.tile(shape, dt, tag=cname, name=f"{cname}_{_cn[0]}")
                nc.sync.dma_start(out=t[:], in_=ap_in)
                return t

            r0h_s = cload(r0h[:, :], [F_IN, HC], BF16)
            r0a_s = cload(r0a[:, :], [F_IN, 8], BF16)
            r1h_s = [cload(r1h[k * 128:(k + 1) * 128, :], [128, HC], BF16)
                     for k in range(KB)]
            r1a_s = [cload(r1a[k * 128:(k + 1) * 128, :], [128, 8], BF16)
                     for k in range(KB)]
            r2_s = [cload(r2[k * 128:(k + 1) * 128, :], [128, FTS]) for k in range(KB)]
            r3_s = cload(r3[:, :], [FTS, 1])
            id_s = cload(ident[:, :], [128, 128])
            idb_s = cload(identb[:, :], [128, 128], BF16)
            srcw_s = cload(srcw[:, :], [128, SW], I16)
            trw_s = cload(trw[:, :], [128, NT * 8], I16)
            ae0_s = cload(ae0[:, :, :], [128, NCHUNK, 4], tag="ae")
            b0_s = cload(b0t[:, :], [128, HC]) if use_b0 else None
            b1_s = cload(b1t[:, :], [128, HC]) if use_b1 else None
            l0b_s = cload(l0bt[:, :], [128, FTS]) if use_l0b else None
            l1b_s = cload(l1bt[:, :], [128, 1]) if use_l1b else None
            xTs = cload(xT[:, :], [F_IN, NP], BF16, tag="xT")

            # persistent alpha_dst for layer-1's own dst tiles (filled in fin0)
            adts1 = adp.tile([128, NT, 4], BF16, tag="adts1")

            # ---- phase A: layer-0 table (xT pre-permuted to row order) ----
            # PSUM alternates between psacc and pstr pools so the PE streams
            # continuously; stores batched in pairs across sync+scalar queues.
            nc._state.push_named_scope("phaseA")
            st2 = None
            for mr in range(MT):
                lx = xTs[:, mr * 128:(mr + 1) * 128]
                pool = psacc if mr % 2 == 0 else pstr
                ph = pool.tile([128, HC], F32, tag="ph" if mr % 2 == 0 else "pt",
                               name="phA")
                nc.tensor.matmul(ph[:], lx, r0h_s[:], start=True, stop=True)
                pa = pss.tile([128, 8], F32, tag="pss")
                nc.tensor.matmul(pa[:], lx, r0a_s[:], start=True, stop=True)
                half = mr % 2
                if half == 0:
                    st2 = stage.tile([128, 2, TW], BF16, tag="hrow2", bufs=4)
                if mr % 2 == 0:
                    nc.vector.tensor_copy(st2[:, half, 0:HC], ph[:])
                    nc.vector.tensor_copy(
                        st2[:, half, HC:HC + 16].bitcast(F32), pa[:])
                else:
                    nc.scalar.activation(st2[:, half, 0:HC], ph[:], AF.Copy)
                    nc.vector.tensor_copy(
                        st2[:, half, HC:HC + 16].bitcast(F32), pa[:])
                if half == 1:
                    dst = H0[(mr - 1) * 128:(mr + 1) * 128, :].rearrange(
                        "(j p) c -> p j c", p=128)
                    eng = nc.sync if (mr // 2) % 2 == 0 else nc.scalar
                    eng.dma_start(out=dst, in_=st2[:])
            nc._state.pop_named_scope("phaseA")

            # ---- layer-0 alpha_dst for own dst tiles: one batched gather ----
            adt_all = adp.tile([128, NT, 128], BF16, tag="adta")
            nc.gpsimd.dma_gather(
                adt_all[:], H0[:, HC:TW], trw_s[:, 0:NT * 8],
                NT * 128, NT * 128, 128, elem_step=TW,
                single_packet=False, queue_num=3)
            adts0 = adp.tile([128, NT, 4], BF16, tag="adts0")
            nc.vector.tensor_copy(adts0[:], adt_all[:, :, 8:16].bitcast(F32))

            # ---- aggregation over one layer's edges ----
            def agg_layer(tbl, ae_s, adts, fin_pre, fin_post):
                gstate = {}

                def nch_of(s):
                    return min(SCC, NCHUNK - s * SCC)

                def issue_gather(s):
                    if s >= NSUP:
                        return
                    nch = nch_of(s)
                    gA = stage.tile([128, SCC, TW], BF16, tag="gA",
                                    name="gA", bufs=4)
                    c0 = s * SCC * 8
                    base = 0
                    nq = min(4, nch)
                    for qi in range(nq):
                        take = (nch - base + (nq - qi) - 1) // (nq - qi)
                        nc.gpsimd.dma_gather(
                            gA[:, base:base + take, :], tbl[:, :],
                            srcw_s[:, c0 + base * 8:c0 + (base + take) * 8],
                            take * 128, take * 128, TW,
                            single_packet=False, queue_num=qi)
                        base += take
                    oz = stage.tile([128, SCC, 256], F8, tag="ohz",
                                    name="oz", bufs=5)
                    nc.sync.dma_start(
                        out=oz[:, 0:nch, :],
                        in_=ohz[:, s * SCC:s * SCC + nch, :])
                    gstate[s] = [gA, oz, None, None]

                def emit_pead(s):
                    if s >= NSUP:
                        return
                    nch = nch_of(s)
                    oz = gstate[s][1]
                    pead = pstr.tile([128, SCC * 4], F32, tag="pt", name="pead")
                    for jj in range(nch):
                        q = s * SCC + jj
                        nc.tensor.matmul(
                            pead[:, jj * 4:(jj + 1) * 4],
                            oz[:, jj, 128:256], adts[:, t_of_q[q], :],
                            start=True, stop=True)
                    gstate[s][2] = pead

                def emit_alpha(s):
                    if s >= NSUP:
                        return
                    nch = nch_of(s)
                    gA, oz, pead, _ = gstate[s]
                    asrc = gA[:, 0:nch, HC:HC + 8].bitcast(F32)
                    t0 = work.tile([128, SCC, 4], F32, tag="t0", bufs=2)
                    nc.vector.tensor_add(
                        t0[:, 0:nch, :], asrc,
                        ae_s[:, s * SCC:s * SCC + nch, :])
                    t1 = work.tile([128, SCC, 4], F32, tag="t1", bufs=2)
                    nc.vector.tensor_add(
                        t1[:, 0:nch, :], t0[:, 0:nch, :],
                        pead[:, 0:nch * 4].rearrange("x (a b) -> x a b", b=4))
                    t2 = work.tile([128, SCC, 4], F32, tag="t2", bufs=2)
                    nc.scalar.activation(
                        t2[:, 0:nch, :], t1[:, 0:nch, :], AF.Copy, scale=0.2)
                    tl = work.tile([128, SCC, 4], F32, tag="tl", bufs=2)
                    nc.vector.tensor_max(
                        tl[:, 0:nch, :], t1[:, 0:nch, :], t2[:, 0:nch, :])
                    pf = work.tile([128, SCC, 4], F32, tag="pf", bufs=3)
                    nc.scalar.activation(pf[:, 0:nch, :], tl[:, 0:nch, :], AF.Exp)
                    pb = work.tile([128, SCC, 4], BF16, tag="p", bufs=3)
                    nc.scalar.activation(pb[:, 0:nch, :], pf[:, 0:nch, :], AF.Copy)
                    gstate[s][3] = (pf, pb)

                issue_gather(0)
                issue_gather(1)
                emit_pead(0)
                emit_alpha(0)

                HV = H - 1  # heads on vector (one 3D op); last head on scalar
                pend = None
                q = 0
                for t in range(NT):
                    ps_o = psacc.tile([128, HC], F32, tag="ph")
                    ps_s = pss.tile([128, 8], F32, tag="pss")
                    fin_t, fin_ar = None, None
                    for k in range(K_t[t]):
                        s, j = divmod(q, SCC)
                        if j == 0:
                            issue_gather(s + 2)
                            emit_pead(s + 1)
                        if j == 2:
                            emit_alpha(s + 1)
                        if pend is not None and k == min(2, K_t[t] - 1):
                            fin_t = pend[0]
                            fin_ar = fin_pre(*pend)
                            pend = None
                        gA, oz, pead, (pf, pb) = gstate[s]
                        gp = work.tile([128, HC], BF16, tag="gp", bufs=4)
                        nc.vector.tensor_mul(
                            gp[:, 0:HV * C].rearrange("x (h c) -> x h c", h=HV),
                            gA[:, j, 0:HV * C].rearrange("x (h c) -> x h c", h=HV),
                            _bcast4(pf[:, :, 0:HV], j, C))
                        nc.scalar.activation(
                            gp[:, HV * C:H * C], gA[:, j, HV * C:H * C],
                            AF.Copy, scale=pf[:, j, HV:HV + 1])
                        first, last = (k == 0), (k == K_t[t] - 1)
                        oh_j = oz[:, j, 0:128]
                        nc.tensor.matmul(ps_o[:], oh_j, gp[:],
                                         start=first, stop=last)
                        nc.tensor.matmul(ps_s[:, 0:4], oh_j, pb[:, j, :],
                                         start=first, stop=last)
                        q += 1
                    if fin_ar is not None:
                        fin_post(fin_t, fin_ar)
                    pend = (t, ps_o, ps_s)
                ar = fin_pre(*pend)
                fin_post(pend[0], ar)

            # ---- tile finalize: softmax-normalize + relu (fused on scalar) --
            def norm_relu(ps_o, ps_s, bias_s, out_dt):
                sp = work.tile([128, 4], F32, tag="sp")
                nc.vector.tensor_scalar_add(sp[:], ps_s[:, 0:4], 1e-16)
                rc = work.tile([128, 4], F32, tag="rc")
                nc.vector.reciprocal(rc[:], sp[:])
                if bias_s is None:
                    ar = work.tile([128, HC], out_dt,
                                   tag=f"ar{out_dt}", bufs=3)
                    for h in range(H):
                        nc.scalar.activation(
                            ar[:, h * C:(h + 1) * C], ps_o[:, h * C:(h + 1) * C],
                            AF.Relu, scale=rc[:, h:h + 1])
                    return ar
                ao = work.tile([128, HC], F32, tag="ao", bufs=2)
                for h in range(H):
                    nc.vector.tensor_scalar_mul(
                        ao[:, h * C:(h + 1) * C], ps_o[:, h * C:(h + 1) * C],
                        rc[:, h:h + 1])
                ab = work.tile([128, HC], F32, tag="ao", bufs=2)
                nc.vector.tensor_add(ab[:], ao[:], bias_s[:])
                ar = work.tile([128, HC], out_dt, tag=f"ar{out_dt}", bufs=3)
                nc.scalar.activation(ar[:], ab[:], AF.Relu)
                return ar

            # ---- layer-0 finalize: transpose + layer-1 linear + group AG ----
            def fin_pre0(t, ps_o, ps_s):
                return norm_relu(ps_o, ps_s, b0_s, BF16)

            def fin_post0(t, ar):
                a0k = []
                for kk in range(KB):
                    pt = pstr.tile([128, 128], BF16, tag="pt", name="ptb")
                    nc.tensor.transpose(pt[:], ar[:, kk * 128:(kk + 1) * 128],
                                        idb_s[:])
                    ak = tp.tile([128, 128], BF16, tag="a1T", name=f"a0k{kk}")
                    nc.vector.tensor_copy(ak[:], pt[:])
                    a0k.append(ak)
                ph1 = psacc.tile([128, HC], F32, tag="ph")
                pa1 = pss.tile([128, 8], F32, tag="pss")
                for kk in range(KB):
                    first, last = (kk == 0), (kk == KB - 1)
                    nc.tensor.matmul(ph1[:], a0k[kk][:], r1h_s[kk][:],
                                     start=first, stop=last)
                    nc.tensor.matmul(pa1[:], a0k[kk][:], r1a_s[kk][:],
                                     start=first, stop=last)
                st = stage.tile([128, TW], BF16, tag="hrow", bufs=4)
                if t % 2 == 0:
                    nc.vector.tensor_copy(st[:, 0:HC], ph1[:])
                else:
                    nc.scalar.activation(st[:, 0:HC], ph1[:], AF.Copy)
                nc.vector.tensor_copy(st[:, HC:HC + 16].bitcast(F32), pa1[:])
                nc.vector.tensor_copy(adts1[:, t, :], pa1[:, 4:8])
                g = grp_of_t[t]
                loc = t - gstart_t[g]
                nc.sync.dma_start(out=H1g[g][loc * 128:(loc + 1) * 128, :],
                                  in_=st[:])
                if loc == GS[g] - 1:
                    r0_, r1_ = (NCORES * 128 * gstart_t[g],
                                NCORES * 128 * gstart_t[g + 1])
                    nc.gpsimd.collective_compute(
                        "AllGather", OP.bypass,
                        replica_groups=[list(range(NCORES))],
                        ins=[H1g[g].opt()],
                        outs=[H1[r0_:r1_, :].opt()],
                    )

            nc._state.push_named_scope("phaseB")
            agg_layer(H0, ae0_s, adts0, fin_pre0, fin_post0)
            nc._state.pop_named_scope("phaseB")

            # ---- layer-1 aggregation + MLP head per dst tile ----
            def fin_pre1(t, ps_o, ps_s):
                return norm_relu(ps_o, ps_s, b1_s, F32)

            def fin_post1(t, ar):
                h2p = psacc.tile([128, FTS], F32, tag="ph")
                for kk in range(KB):
                    pt = pstr.tile([128, 128], F32, tag="pt", name="ptf")
                    nc.tensor.transpose(pt[:], ar[:, kk * 128:(kk + 1) * 128],
                                        id_s[:])
                    a1k = tp.tile([128, 128], F32, tag="a1T")
                    nc.vector.tensor_copy(a1k[:], pt[:])
                    nc.tensor.matmul(h2p[:], a1k[:], r2_s[kk][:],
                                     start=(kk == 0), stop=(kk == KB - 1))
                if use_l0b:
                    h2b = work.tile([128, FTS], F32, tag="h2b")
                    nc.vector.tensor_add(h2b[:], h2p[:], l0b_s[:])
                else:
                    h2b = h2p
                h2r = work.tile([128, FTS], F32, tag="h2r")
                nc.scalar.activation(h2r[:], h2b[:], AF.Relu)
                pt2 = pstr.tile([128, 128], F32, tag="pt")
                nc.tensor.transpose(pt2[:], h2r[:], id_s[:])
                h2T = tp.tile([128, 128], F32, tag="a1T")
                nc.vector.tensor_copy(h2T[:], pt2[:])
                po = pss.tile([128, 8], F32, tag="pss")
                nc.tensor.matmul(po[:, 0:1], h2T[:], r3_s[:],
                                 start=True, stop=True)
                ob = work.tile([128, 1], F32, tag="ob")
                if use_l1b:
                    nc.vector.tensor_add(ob[:], po[:, 0:1], l1b_s[:])
                else:
                    nc.vector.tensor_copy(ob[:], po[:, 0:1])
                nc.sync.dma_start(out=outp[t * 128:(t + 1) * 128, :], in_=ob[:])

            nc._state.push_named_scope("phaseD")
            ae1_s = cload(ae1[:, :, :], [128, NCHUNK, 4], tag="ae")
            agg_layer(H1, ae1_s, adts1, fin_pre1, fin_post1)
            nc._state.pop_named_scope("phaseD")

    nc.finalize()
    return nc


def _wrap_idx(v, E_pad):
    blk = np.zeros((16, E_pad // 16), np.int16)
    ar = np.arange(E_pad)
    blk[ar % 16, ar // 16] = v.astype(np.int16)
    return np.tile(blk, (8, 1))


def kernel(x, edge_index, edge_weights,
           W0, as0, ad0, We0, ae0, b0,
           W1, as1, ad1, We1, ae1, b1,
           L0W, L0b, L1W, L1b):
    x = np.asarray(x, np.float32)
    N, F_IN = x.shape
    HC = W0.shape[0]
    H, C = np.asarray(as0).shape
    FTS = np.asarray(L0W).shape[0]

    NT = -(-N // (128 * NCORES))
    SHARD = NT * 128
    NP = SHARD * NCORES

    # AllGather group sizes (in dst tiles): big groups early (lots of overlap
    # time), shrinking tail so the last collective is small.
    if NT == 20:
        GS = [6, 5, 4, 2, 2, 1]
    else:
        GS = []
        rem = NT
        while rem > 0:
            s = max(1, -(-rem // 3))
            GS.append(min(s, rem))
            rem -= GS[-1]
    gstart_t = np.concatenate([[0], np.cumsum(GS)]).astype(np.int64)
    grp_of_t = np.zeros(NT, np.int64)
    for g, s in enumerate(GS):
        grp_of_t[gstart_t[g]:gstart_t[g + 1]] = g
    GS_arr = np.asarray(GS, np.int64)

    # table-row permutation (group-major) so group AllGathers land contiguous
    nodes = np.arange(NP)
    core = nodes // SHARD
    rr = nodes % SHARD
    tt = rr // 128
    gg = grp_of_t[tt]
    off = (tt - gstart_t[gg]) * 128 + rr % 128
    t_of_n = (NCORES * 128 * gstart_t[gg] + core * (GS_arr[gg] * 128)
              + off)                                   # node -> table row

    # ---- edges ----
    ew_in = np.asarray(edge_weights, np.float32)
    src = np.concatenate([np.asarray(edge_index[0]), np.arange(N)])
    dst = np.concatenate([np.asarray(edge_index[1]), np.arange(N)])
    ew = np.concatenate([ew_in, np.full(N, ew_in.mean(), np.float32)])
    order = np.argsort(dst, kind="stable")
    src_s, dst_s, ew_s = src[order], dst[order], ew[order]

    NTG = NP // 128
    tile_of = (dst_s // 128).astype(np.int64)
    tcounts = np.bincount(tile_of, minlength=NTG)
    tstart = np.concatenate([[0], np.cumsum(tcounts)])

    K_t = [max(1, int(max(-(-tcounts[i * NT + t] // 128) for i in range(NCORES))))
           for t in range(NT)]
    NCHUNK = int(sum(K_t))
    E_pad = NCHUNK * 128

    # ---- weight folding (host, O(weights)) ----
    as0 = np.asarray(as0, np.float32)
    ad0 = np.asarray(ad0, np.float32)
    ae0w = np.asarray(ae0, np.float32)
    as1 = np.asarray(as1, np.float32)
    ad1 = np.asarray(ad1, np.float32)
    ae1w = np.asarray(ae1, np.float32)
    W0 = np.asarray(W0, np.float32)
    W1 = np.asarray(W1, np.float32)
    We0 = np.asarray(We0, np.float32)
    We1 = np.asarray(We1, np.float32)

    k0 = (We0.reshape(H, C) * ae0w).sum(1).astype(np.float32)
    k1 = (We1.reshape(H, C) * ae1w).sum(1).astype(np.float32)

    def fold(W, a):
        blk = np.zeros((HC, H), np.float32)
        for h in range(H):
            blk[h * C:(h + 1) * C, h] = a[h]
        return (W.T @ blk).astype(np.float32)

    bf = ml_dtypes.bfloat16
    r0h = W0.T.astype(bf)
    r0a = np.concatenate([fold(W0, as0), fold(W0, ad0)], 1).astype(bf)
    r1h = W1.T.astype(bf)
    r1a = np.concatenate([fold(W1, as1), fold(W1, ad1)], 1).astype(bf)
    r2 = np.asarray(L0W, np.float32).T.copy()
    r3 = np.asarray(L1W, np.float32).T.copy()

    # xT in TABLE-ROW order: column r of xT = x[node(r)]
    inv = np.empty(NP, np.int64)
    inv[t_of_n] = nodes                              # table row -> node
    xa = np.zeros((NP, F_IN), np.float32)
    xa[:N] = x
    xT = np.ascontiguousarray(xa[inv].T).astype(bf)

    ident = np.eye(128, dtype=np.float32)
    identb = np.eye(128, dtype=np.float32).astype(bf)

    use_b0 = bool(np.any(b0))
    use_b1 = bool(np.any(b1))
    use_l0b = bool(np.any(np.asarray(L0b)))
    use_l1b = bool(np.any(np.asarray(L1b)))

    in_maps = []
    for i in range(NCORES):
        srcp = np.zeros(E_pad, np.int64)
        dlocp = np.full(E_pad, -1, np.int64)
        ewp = np.zeros(E_pad, np.float32)
        offq = 0
        for t in range(NT):
            g = i * NT + t
            cnt = int(tcounts[g])
            sl = slice(tstart[g], tstart[g] + cnt)
            srcp[offq:offq + cnt] = t_of_n[src_s[sl]]
            dlocp[offq:offq + cnt] = dst_s[sl] - g * 128
            ewp[offq:offq + cnt] = ew_s[sl]
            offq += K_t[t] * 128
        ae0p = (ewp[:, None] * k0[None, :]).reshape(NCHUNK, 128, 4).transpose(1, 0, 2)
        ae1p = (ewp[:, None] * k1[None, :]).reshape(NCHUNK, 128, 4).transpose(1, 0, 2)
        # one-hot blocks: ohz[e, q, d] = ohb, ohz[d, q, 128+e] = oht
        ohcube = np.zeros((NCHUNK, 128, 128), np.float32)  # [q, e, d]
        dl2 = dlocp.reshape(NCHUNK, 128)
        valid = dl2 >= 0
        qs, es = np.nonzero(valid)
        ohcube[qs, es, dl2[qs, es]] = 1.0
        f8 = ml_dtypes.float8_e4m3fn
        ohz_np = np.empty((128, NCHUNK, 256), f8)
        ohz_np[:, :, 0:128] = ohcube.transpose(1, 0, 2).astype(f8)
        ohz_np[:, :, 128:256] = ohcube.transpose(2, 0, 1).astype(f8)
        # own dst-tile table rows for the layer-0 alpha_dst gather
        trows = np.empty((NT, 128), np.int64)
        for t in range(NT):
            base = t_of_n[i * SHARD + t * 128]
            trows[t] = base + np.arange(128)
        trw_np = _wrap_idx(trows.reshape(-1), NT * 128)
        im = {
            "xT": xT, "r0h": r0h, "r0a": r0a, "r1h": r1h, "r1a": r1a,
            "r2": r2, "r3": r3, "ident": ident, "identb": identb,
            "srcw": _wrap_idx(srcp, E_pad), "trw": trw_np,
            "ohz": ohz_np,
            "ae0": np.ascontiguousarray(ae0p),
            "ae1": np.ascontiguousarray(ae1p),
        }
        if use_b0:
            im["b0t"] = np.tile(np.asarray(b0, np.float32)[None, :], (128, 1))
        if use_b1:
            im["b1t"] = np.tile(np.asarray(b1, np.float32)[None, :], (128, 1))
        if use_l0b:
            im["l0bt"] = np.tile(np.asarray(L0b, np.float32)[None, :], (128, 1))
        if use_l1b:
            im["l1bt"] = np.tile(np.asarray(L1b, np.float32).reshape(1, 1), (128, 1))
        in_maps.append(im)

    nc = _build_program(NP, F_IN, HC, H, C, NT, K_t, FTS, GS,
                        use_b0, use_b1, use_l0b, use_l1b)
    res = run_bass_kernel_spmd(nc, in_maps, list(range(NCORES)))
    out = np.concatenate([res.results[i]["out"][:, 0] for i in range(NCORES)])
    return out[:N].astype(np.float32)


# revision 28
# speedup vs baseline: 1.6964x; 1.0235x over previous
"""2-layer GAT + MLP head on 8 TRN2 NeuronCores.

Strategy (dst-sharded, software-pipelined):
- Nodes padded to NP=20480; each core owns a contiguous 2560-dst shard.
- Edges (incl. self-loops, PyG mean-fill edge attr) sorted by dst,
  grouped into 128-dst tiles, padded per tile-slot to a chunk count K_t
  shared by all cores (SPMD: one program).
- Per layer a node table [NP, 640] bf16 in HBM: cols [0:512) = h,
  bytes [1024:1056) = asrc|adst (f32 bits). Rows permuted group-major
  so layer-1 tables assemble from per-group AllGathers (Shared HBM).
- Aggregation per 128-edge chunk: gather rows by src (1 DMA per
  8-chunk super), one-hot blocks ([e,d] bf16 + [d,e] bf16) streamed as
  one fused ohz tensor; p = exp(lrelu(asrc+adst+aedge)) with adst
  expanded via one-hot matmul; out[dst] += (p*h) via one-hot matmul in
  PSUM; denominator via second matmul with rhs=p.
- Pipelining: gathers issued 2 supers ahead, alpha chains 1 super
  ahead, tile finalize deferred by 1 tile so the in-order PE stream
  never waits on vector/scalar chains.
- dst-tile alphas for layer 1 captured into SBUF during fin0 (no
  gather); layer-0 ones via one batched 2560-row gather from H0.
"""

import numpy as np
import ml_dtypes

import concourse.bacc as bacc
import concourse.bass as bass
import concourse.mybir as mybir
import concourse.tile as tile
from concourse.bass_utils import run_bass_kernel_spmd

F32 = mybir.dt.float32
F8 = mybir.dt.float8e4
BF16 = mybir.dt.bfloat16
I16 = mybir.dt.int16
AF = mybir.ActivationFunctionType
OP = mybir.AluOpType

NCORES = 8
SCC = 8  # chunks (of 128 edges) per gather super-chunk


def _bcast4(ap_tile, j, reps):
    """[128, SCC, 4] tile -> [128, 4, reps] zero-step broadcast AP of slot j."""
    sl = ap_tile[:, j, :]
    return bass.AP(sl.tensor, sl.offset, [list(sl.ap[0]), list(sl.ap[-1]), [0, reps]])


def _bcastI(ap_tile, j, reps):
    """[128, SCC, 4] tile -> [128, reps, 4] broadcast AP of slot j with the
    head dim last at unit stride (head-interleaved gp layout)."""
    sl = ap_tile[:, j, :]
    return bass.AP(sl.tensor, sl.offset, [list(sl.ap[0]), [0, reps], list(sl.ap[-1])])


def _build_program(NP, F_IN, HC, H, C, NT, K_t, FTS, GS, use_b0, use_b1,
                   use_l0b, use_l1b):
    NCHUNK = int(sum(K_t))
    E_pad = NCHUNK * 128
    SW = E_pad // 16
    TW = HC + 128  # bf16 table row: h | asrc,adst (f32 bits) | pad
    KB = HC // 128
    NAG = len(GS)            # allgather groups (variable tile counts)
    gstart_t = [0]
    for s_ in GS:
        gstart_t.append(gstart_t[-1] + s_)
    grp_of_t = []
    for g, s_ in enumerate(GS):
        grp_of_t += [g] * s_
    NSUP = -(-NCHUNK // SCC)
    MT = NP // 128

    # chunk q -> owning dst tile
    t_of_q = []
    for t in range(NT):
        t_of_q += [t] * K_t[t]

    nc = bacc.Bacc(dynamic_dma_scratch_size=65536, num_swdge_queues=4)
    P = nc.declare_dram_parameter

    xT = P("xT", [F_IN, NP], BF16, isOutput=False)
    r0h = P("r0h", [F_IN, HC], BF16, isOutput=False)
    r0a = P("r0a", [F_IN, 8], BF16, isOutput=False)
    r1h = P("r1h", [HC, HC], BF16, isOutput=False)
    r1a = P("r1a", [HC, 8], BF16, isOutput=False)
    r2 = P("r2", [HC, FTS], F32, isOutput=False)
    r3 = P("r3", [FTS, 1], F32, isOutput=False)
    ident = P("ident", [128, 128], F32, isOutput=False)
    identb = P("identb", [128, 128], BF16, isOutput=False)
    srcw = P("srcw", [128, SW], I16, isOutput=False)
    trw = P("trw", [128, NT * 8], I16, isOutput=False)
    ohz = P("ohz", [128, NCHUNK, 256], F8, isOutput=False)
    ae0 = P("ae0", [128, NCHUNK, 4], F32, isOutput=False)
    ae1 = P("ae1", [128, NCHUNK, 4], F32, isOutput=False)
    if use_b0:
        b0t = P("b0t", [128, HC], F32, isOutput=False)
    if use_b1:
        b1t = P("b1t", [128, HC], F32, isOutput=False)
    if use_l0b:
        l0bt = P("l0bt", [128, FTS], F32, isOutput=False)
    if use_l1b:
        l1bt = P("l1bt", [128, 1], F32, isOutput=False)
    outp = P("out", [NT * 128, 1], F32, isOutput=True)

    with tile.TileContext(nc) as tc:
        with (
            tc.tile_pool(name="const", bufs=1) as const,
            tc.tile_pool(name="stage", bufs=2) as stage,
            tc.tile_pool(name="work", bufs=3) as work,
            tc.tile_pool(name="tp", bufs=6) as tp,
            tc.tile_pool(name="adp", bufs=1) as adp,
            tc.tile_pool(name="psacc", bufs=3, space="PSUM") as psacc,
            tc.tile_pool(name="pss", bufs=2, space="PSUM") as pss,
            tc.tile_pool(name="pstr", bufs=3, space="PSUM") as pstr,
            tc.tile_pool(name="dram", bufs=1, space="DRAM") as dram,
        ):
            H0 = dram.tile([NP, TW], BF16, tag="H0")
            H1 = dram.tile([NP, TW], BF16, tag="H1")
            H1g = [dram.tile([GS[g] * 128, TW], BF16, tag=f"H1g{g}",
                             name=f"H1g{g}") for g in range(NAG)]

            _cn = [0]

            def cload(ap_in, shape, dt=F32, tag=None):
                _cn[0] += 1
                cname = tag or f"c{_cn[0]}"
                t = const.tile(shape, dt, tag=cname, name=f"{cname}_{_cn[0]}")
                nc.sync.dma_start(out=t[:], in_=ap_in)
                return t

            r0h_s = cload(r0h[:, :], [F_IN, HC], BF16)
            r0a_s = cload(r0a[:, :], [F_IN, 8], BF16)
            r1h_s = [cload(r1h[k * 128:(k + 1) * 128, :], [128, HC], BF16)
                     for k in range(KB)]
            r1a_s = [cload(r1a[k * 128:(k + 1) * 128, :], [128, 8], BF16)
                     for k in range(KB)]
            r2_s = [cload(r2[k * 128:(k + 1) * 128, :], [128, FTS]) for k in range(KB)]
            r3_s = cload(r3[:, :], [FTS, 1])
            id_s = cload(ident[:, :], [128, 128])
            idb_s = cload(identb[:, :], [128, 128], BF16)
            srcw_s = cload(srcw[:, :], [128, SW], I16)
            trw_s = cload(trw[:, :], [128, NT * 8], I16)
            ae0_s = cload(ae0[:, :, :], [128, NCHUNK, 4], tag="ae")
            b0_s = cload(b0t[:, :], [128, HC]) if use_b0 else None
            b1_s = cload(b1t[:, :], [128, HC]) if use_b1 else None
            l0b_s = cload(l0bt[:, :], [128, FTS]) if use_l0b else None
            l1b_s = cload(l1bt[:, :], [128, 1]) if use_l1b else None
            xTs = cload(xT[:, :], [F_IN, NP], BF16, tag="xT")

            # persistent alpha_dst for layer-1's own dst tiles (filled in fin0)
            adts1 = adp.tile([128, NT, 4], BF16, tag="adts1")

            # ---- phase A: layer-0 table (xT pre-permuted to row order) ----
            # PSUM alternates between psacc and pstr pools so the PE streams
            # continuously; stores batched in pairs across sync+scalar queues.
            nc._state.push_named_scope("phaseA")
            st2 = None
            for mr in range(MT):
                lx = xTs[:, mr * 128:(mr + 1) * 128]
                pool = psacc if mr % 2 == 0 else pstr
                ph = pool.tile([128, HC], F32, tag="ph" if mr % 2 == 0 else "pt",
                               name="phA")
                nc.tensor.matmul(ph[:], lx, r0h_s[:], start=True, stop=True)
                pa = pss.tile([128, 8], F32, tag="pss")
                nc.tensor.matmul(pa[:], lx, r0a_s[:], start=True, stop=True)
                half = mr % 2
                if half == 0:
                    st2 = stage.tile([128, 2, TW], BF16, tag="hrow2", bufs=4)
                if mr % 2 == 0:
                    nc.vector.tensor_copy(st2[:, half, 0:HC], ph[:])
                    nc.vector.tensor_copy(
                        st2[:, half, HC:HC + 16].bitcast(F32), pa[:])
                else:
                    nc.scalar.activation(st2[:, half, 0:HC], ph[:], AF.Copy)
                    nc.vector.tensor_copy(
                        st2[:, half, HC:HC + 16].bitcast(F32), pa[:])
                if half == 1:
                    dst = H0[(mr - 1) * 128:(mr + 1) * 128, :].rearrange(
                        "(j p) c -> p j c", p=128)
                    eng = nc.sync if (mr // 2) % 2 == 0 else nc.scalar
                    eng.dma_start(out=dst, in_=st2[:])
            nc._state.pop_named_scope("phaseA")

            # ---- layer-0 alpha_dst for own dst tiles: one batched gather ----
            adt_all = adp.tile([128, NT, 128], BF16, tag="adta")
            nc.gpsimd.dma_gather(
                adt_all[:], H0[:, HC:TW], trw_s[:, 0:NT * 8],
                NT * 128, NT * 128, 128, elem_step=TW,
                single_packet=False, queue_num=3)
            adts0 = adp.tile([128, NT, 4], BF16, tag="adts0")
            nc.vector.tensor_copy(adts0[:], adt_all[:, :, 8:16].bitcast(F32))

            # ---- aggregation over one layer's edges ----
            def agg_layer(tbl, ae_s, adts, fin_pre, fin_post):
                gstate = {}

                def nch_of(s):
                    return min(SCC, NCHUNK - s * SCC)

                def issue_gather(s):
                    if s >= NSUP:
                        return
                    nch = nch_of(s)
                    gA = stage.tile([128, SCC, TW], BF16, tag="gA",
                                    name="gA", bufs=4)
                    c0 = s * SCC * 8
                    base = 0
                    nq = min(4, nch)
                    for qi in range(nq):
                        take = (nch - base + (nq - qi) - 1) // (nq - qi)
                        nc.gpsimd.dma_gather(
                            gA[:, base:base + take, :], tbl[:, :],
                            srcw_s[:, c0 + base * 8:c0 + (base + take) * 8],
                            take * 128, take * 128, TW,
                            single_packet=False, queue_num=qi)
                        base += take
                    oz = stage.tile([128, SCC, 256], F8, tag="ohz",
                                    name="oz", bufs=5)
                    nc.sync.dma_start(
                        out=oz[:, 0:nch, :],
                        in_=ohz[:, s * SCC:s * SCC + nch, :])
                    gstate[s] = [gA, oz, None, None]

                def emit_pead(s):
                    if s >= NSUP:
                        return
                    nch = nch_of(s)
                    oz = gstate[s][1]
                    pead = pstr.tile([128, SCC * 4], F32, tag="pt", name="pead")
                    for jj in range(nch):
                        q = s * SCC + jj
                        nc.tensor.matmul(
                            pead[:, jj * 4:(jj + 1) * 4],
                            oz[:, jj, 128:256], adts[:, t_of_q[q], :],
                            start=True, stop=True)
                    gstate[s][2] = pead

                def emit_alpha(s):
                    if s >= NSUP:
                        return
                    nch = nch_of(s)
                    gA, oz, pead, _ = gstate[s]
                    asrc = gA[:, 0:nch, HC:HC + 8].bitcast(F32)
                    t0 = work.tile([128, SCC, 4], F32, tag="t0", bufs=2)
                    nc.vector.tensor_add(
                        t0[:, 0:nch, :], asrc,
                        ae_s[:, s * SCC:s * SCC + nch, :])
                    t1 = work.tile([128, SCC, 4], F32, tag="t1", bufs=2)
                    nc.vector.tensor_add(
                        t1[:, 0:nch, :], t0[:, 0:nch, :],
                        pead[:, 0:nch * 4].rearrange("x (a b) -> x a b", b=4))
                    t2 = work.tile([128, SCC, 4], F32, tag="t2", bufs=2)
                    nc.scalar.activation(
                        t2[:, 0:nch, :], t1[:, 0:nch, :], AF.Copy, scale=0.2)
                    tl = work.tile([128, SCC, 4], F32, tag="tl", bufs=2)
                    nc.vector.tensor_max(
                        tl[:, 0:nch, :], t1[:, 0:nch, :], t2[:, 0:nch, :])
                    pf = work.tile([128, SCC, 4], F32, tag="pf", bufs=3)
                    nc.scalar.activation(pf[:, 0:nch, :], tl[:, 0:nch, :], AF.Exp)
                    pb = work.tile([128, SCC, 4], BF16, tag="p", bufs=3)
                    nc.scalar.activation(pb[:, 0:nch, :], pf[:, 0:nch, :], AF.Copy)
                    gstate[s][3] = (pf, pb)

                issue_gather(0)
                issue_gather(1)
                emit_pead(0)
                emit_alpha(0)

                pend = None
                q = 0
                for t in range(NT):
                    ps_o = psacc.tile([128, HC], F32, tag="ph")
                    ps_s = pss.tile([128, 8], F32, tag="pss")
                    fin_t, fin_ar = None, None
                    for k in range(K_t[t]):
                        s, j = divmod(q, SCC)
                        if j == 0:
                            issue_gather(s + 2)
                            emit_pead(s + 1)
                        if j == 2:
                            emit_alpha(s + 1)
                        if pend is not None and k == min(2, K_t[t] - 1):
                            fin_t = pend[0]
                            fin_ar = fin_pre(*pend)
                            pend = None
                        gA, oz, pead, (pf, pb) = gstate[s]
                        gp = work.tile([128, HC], BF16, tag="gp", bufs=4)
                        nc.vector.tensor_mul(
                            gp[:].rearrange("x (c h) -> x c h", h=H),
                            gA[:, j, 0:HC].rearrange("x (c h) -> x c h", h=H),
                            _bcastI(pb, j, C))
                        first, last = (k == 0), (k == K_t[t] - 1)
                        oh_j = oz[:, j, 0:128]
                        nc.tensor.matmul(ps_o[:], oh_j, gp[:],
                                         start=first, stop=last)
                        nc.tensor.matmul(ps_s[:, 0:4], oh_j, pb[:, j, :],
                                         start=first, stop=last)
                        q += 1
                    if fin_ar is not None:
                        fin_post(fin_t, fin_ar)
                    pend = (t, ps_o, ps_s)
                ar = fin_pre(*pend)
                fin_post(pend[0], ar)

            # ---- tile finalize: softmax-normalize + relu (fused on scalar) --
            def norm_relu(ps_o, ps_s, bias_s, out_dt):
                sp = work.tile([128, 4], F32, tag="sp")
                nc.vector.tensor_scalar_add(sp[:], ps_s[:, 0:4], 1e-16)
                rc = work.tile([128, 4], F32, tag="rc")
                nc.vector.reciprocal(rc[:], sp[:])
                if bias_s is None:
                    ar = work.tile([128, HC], out_dt,
                                   tag=f"ar{out_dt}", bufs=3)
                    arv = ar[:].rearrange("x (c h) -> x h c", h=H)
                    psv = ps_o[:].rearrange("x (c h) -> x h c", h=H)
                    for h in range(H):
                        nc.scalar.activation(
                            arv[:, h, :], psv[:, h, :],
                            AF.Relu, scale=rc[:, h:h + 1])
                    return ar
                ao = work.tile([128, HC], F32, tag="ao", bufs=2)
                aov = ao[:].rearrange("x (c h) -> x h c", h=H)
                psv = ps_o[:].rearrange("x (c h) -> x h c", h=H)
                for h in range(H):
                    nc.vector.tensor_scalar_mul(
                        aov[:, h, :], psv[:, h, :], rc[:, h:h + 1])
                ab = work.tile([128, HC], F32, tag="ao", bufs=2)
                nc.vector.tensor_add(ab[:], ao[:], bias_s[:])
                ar = work.tile([128, HC], out_dt, tag=f"ar{out_dt}", bufs=3)
                nc.scalar.activation(ar[:], ab[:], AF.Relu)
                return ar

            # ---- layer-0 finalize: transpose + layer-1 linear + group AG ----
            def fin_pre0(t, ps_o, ps_s):
                return norm_relu(ps_o, ps_s, b0_s, BF16)

            def fin_post0(t, ar):
                a0k = []
                for kk in range(KB):
                    pt = pstr.tile([128, 128], BF16, tag="pt", name="ptb")
                    nc.tensor.transpose(pt[:], ar[:, kk * 128:(kk + 1) * 128],
                                        idb_s[:])
                    ak = tp.tile([128, 128], BF16, tag="a1T", name=f"a0k{kk}")
                    if kk % 2 == 0:
                        nc.scalar.activation(ak[:], pt[:], AF.Copy)
                    else:
                        nc.vector.tensor_copy(ak[:], pt[:])
                    a0k.append(ak)
                ph1 = psacc.tile([128, HC], F32, tag="ph")
                pa1 = pss.tile([128, 8], F32, tag="pss")
                for kk in range(KB):
                    first, last = (kk == 0), (kk == KB - 1)
                    nc.tensor.matmul(ph1[:], a0k[kk][:], r1h_s[kk][:],
                                     start=first, stop=last)
                    nc.tensor.matmul(pa1[:], a0k[kk][:], r1a_s[kk][:],
                                     start=first, stop=last)
                st = stage.tile([128, TW], BF16, tag="hrow", bufs=4)
                if t % 2 == 0:
                    nc.vector.tensor_copy(st[:, 0:HC], ph1[:])
                else:
                    nc.scalar.activation(st[:, 0:HC], ph1[:], AF.Copy)
                nc.vector.tensor_copy(st[:, HC:HC + 16].bitcast(F32), pa1[:])
                nc.vector.tensor_copy(adts1[:, t, :], pa1[:, 4:8])
                g = grp_of_t[t]
                loc = t - gstart_t[g]
                nc.sync.dma_start(out=H1g[g][loc * 128:(loc + 1) * 128, :],
                                  in_=st[:])
                if loc == GS[g] - 1:
                    r0_, r1_ = (NCORES * 128 * gstart_t[g],
                                NCORES * 128 * gstart_t[g + 1])
                    nc.gpsimd.collective_compute(
                        "AllGather", OP.bypass,
                        replica_groups=[list(range(NCORES))],
                        ins=[H1g[g].opt()],
                        outs=[H1[r0_:r1_, :].opt()],
                    )

            nc._state.push_named_scope("phaseB")
            agg_layer(H0, ae0_s, adts0, fin_pre0, fin_post0)
            nc._state.pop_named_scope("phaseB")

            # ---- layer-1 aggregation + MLP head per dst tile ----
            def fin_pre1(t, ps_o, ps_s):
                return norm_relu(ps_o, ps_s, b1_s, F32)

            def fin_post1(t, ar):
                h2p = psacc.tile([128, FTS], F32, tag="ph")
                for kk in range(KB):
                    pt = pstr.tile([128, 128], F32, tag="pt", name="ptf")
                    nc.tensor.transpose(pt[:], ar[:, kk * 128:(kk + 1) * 128],
                                        id_s[:])
                    a1k = tp.tile([128, 128], F32, tag="a1T")
                    nc.vector.tensor_copy(a1k[:], pt[:])
                    nc.tensor.matmul(h2p[:], a1k[:], r2_s[kk][:],
                                     start=(kk == 0), stop=(kk == KB - 1))
                if use_l0b:
                    h2b = work.tile([128, FTS], F32, tag="h2b")
                    nc.vector.tensor_add(h2b[:], h2p[:], l0b_s[:])
                else:
                    h2b = h2p
                h2r = work.tile([128, FTS], F32, tag="h2r")
                nc.scalar.activation(h2r[:], h2b[:], AF.Relu)
                pt2 = pstr.tile([128, 128], F32, tag="pt")
                nc.tensor.transpose(pt2[:], h2r[:], id_s[:])
                h2T = tp.tile([128, 128], F32, tag="a1T")
                nc.vector.tensor_copy(h2T[:], pt2[:])
                po = pss.tile([128, 8], F32, tag="pss")
                nc.tensor.matmul(po[:, 0:1], h2T[:], r3_s[:],
                                 start=True, stop=True)
                ob = work.tile([128, 1], F32, tag="ob")
                if use_l1b:
                    nc.vector.tensor_add(ob[:], po[:, 0:1], l1b_s[:])
                else:
                    nc.vector.tensor_copy(ob[:], po[:, 0:1])
                nc.sync.dma_start(out=outp[t * 128:(t + 1) * 128, :], in_=ob[:])

            nc._state.push_named_scope("phaseD")
            ae1_s = cload(ae1[:, :, :], [128, NCHUNK, 4], tag="ae")
            agg_layer(H1, ae1_s, adts1, fin_pre1, fin_post1)
            nc._state.pop_named_scope("phaseD")

    nc.finalize()
    return nc


def _wrap_idx(v, E_pad):
    blk = np.zeros((16, E_pad // 16), np.int16)
    ar = np.arange(E_pad)
    blk[ar % 16, ar // 16] = v.astype(np.int16)
    return np.tile(blk, (8, 1))


def kernel(x, edge_index, edge_weights,
           W0, as0, ad0, We0, ae0, b0,
           W1, as1, ad1, We1, ae1, b1,
           L0W, L0b, L1W, L1b):
    x = np.asarray(x, np.float32)
    N, F_IN = x.shape
    HC = W0.shape[0]
    H, C = np.asarray(as0).shape
    FTS = np.asarray(L0W).shape[0]

    NT = -(-N // (128 * NCORES))
    SHARD = NT * 128
    NP = SHARD * NCORES

    # AllGather group sizes (in dst tiles): big groups early (lots of overlap
    # time), shrinking tail so the last collective is small.
    if NT == 20:
        GS = [6, 5, 4, 2, 2, 1]
    else:
        GS = []
        rem = NT
        while rem > 0:
            s = max(1, -(-rem // 3))
            GS.append(min(s, rem))
            rem -= GS[-1]
    gstart_t = np.concatenate([[0], np.cumsum(GS)]).astype(np.int64)
    grp_of_t = np.zeros(NT, np.int64)
    for g, s in enumerate(GS):
        grp_of_t[gstart_t[g]:gstart_t[g + 1]] = g
    GS_arr = np.asarray(GS, np.int64)

    # table-row permutation (group-major) so group AllGathers land contiguous
    nodes = np.arange(NP)
    core = nodes // SHARD
    rr = nodes % SHARD
    tt = rr // 128
    gg = grp_of_t[tt]
    off = (tt - gstart_t[gg]) * 128 + rr % 128
    t_of_n = (NCORES * 128 * gstart_t[gg] + core * (GS_arr[gg] * 128)
              + off)                                   # node -> table row

    # ---- edges ----
    ew_in = np.asarray(edge_weights, np.float32)
    src = np.concatenate([np.asarray(edge_index[0]), np.arange(N)])
    dst = np.concatenate([np.asarray(edge_index[1]), np.arange(N)])
    ew = np.concatenate([ew_in, np.full(N, ew_in.mean(), np.float32)])
    order = np.argsort(dst, kind="stable")
    src_s, dst_s, ew_s = src[order], dst[order], ew[order]

    NTG = NP // 128
    tile_of = (dst_s // 128).astype(np.int64)
    tcounts = np.bincount(tile_of, minlength=NTG)
    tstart = np.concatenate([[0], np.cumsum(tcounts)])

    K_t = [max(1, int(max(-(-tcounts[i * NT + t] // 128) for i in range(NCORES))))
           for t in range(NT)]
    NCHUNK = int(sum(K_t))
    E_pad = NCHUNK * 128

    # ---- weight folding (host, O(weights)) ----
    as0 = np.asarray(as0, np.float32)
    ad0 = np.asarray(ad0, np.float32)
    ae0w = np.asarray(ae0, np.float32)
    as1 = np.asarray(as1, np.float32)
    ad1 = np.asarray(ad1, np.float32)
    ae1w = np.asarray(ae1, np.float32)
    W0 = np.asarray(W0, np.float32)
    W1 = np.asarray(W1, np.float32)
    We0 = np.asarray(We0, np.float32)
    We1 = np.asarray(We1, np.float32)

    k0 = (We0.reshape(H, C) * ae0w).sum(1).astype(np.float32)
    k1 = (We1.reshape(H, C) * ae1w).sum(1).astype(np.float32)

    def fold(W, a):
        blk = np.zeros((HC, H), np.float32)
        for h in range(H):
            blk[h * C:(h + 1) * C, h] = a[h]
        return (W.T @ blk).astype(np.float32)

    bf = ml_dtypes.bfloat16
    # head-interleaved column order: table col c*H+h holds head h channel c,
    # so the per-edge p broadcast is unit-stride on the last dim (DVE 2X).
    iperm = np.asarray([h * C + c for c in range(C) for h in range(H)])
    r0h = W0.T[:, iperm].astype(bf)
    r0a = np.concatenate([fold(W0, as0), fold(W0, ad0)], 1).astype(bf)
    r1h = W1.T[iperm][:, iperm].astype(bf)
    r1a = np.concatenate([fold(W1, as1), fold(W1, ad1)], 1)[iperm].astype(bf)
    r2 = np.asarray(L0W, np.float32).T[iperm].copy()
    r3 = np.asarray(L1W, np.float32).T.copy()

    # xT in TABLE-ROW order: column r of xT = x[node(r)]
    inv = np.empty(NP, np.int64)
    inv[t_of_n] = nodes                              # table row -> node
    xa = np.zeros((NP, F_IN), np.float32)
    xa[:N] = x
    xT = np.ascontiguousarray(xa[inv].T).astype(bf)

    ident = np.eye(128, dtype=np.float32)
    identb = np.eye(128, dtype=np.float32).astype(bf)

    use_b0 = bool(np.any(b0))
    use_b1 = bool(np.any(b1))
    use_l0b = bool(np.any(np.asarray(L0b)))
    use_l1b = bool(np.any(np.asarray(L1b)))

    in_maps = []
    for i in range(NCORES):
        srcp = np.zeros(E_pad, np.int64)
        dlocp = np.full(E_pad, -1, np.int64)
        ewp = np.zeros(E_pad, np.float32)
        offq = 0
        for t in range(NT):
            g = i * NT + t
            cnt = int(tcounts[g])
            sl = slice(tstart[g], tstart[g] + cnt)
            srcp[offq:offq + cnt] = t_of_n[src_s[sl]]
            dlocp[offq:offq + cnt] = dst_s[sl] - g * 128
            ewp[offq:offq + cnt] = ew_s[sl]
            offq += K_t[t] * 128
        ae0p = (ewp[:, None] * k0[None, :]).reshape(NCHUNK, 128, 4).transpose(1, 0, 2)
        ae1p = (ewp[:, None] * k1[None, :]).reshape(NCHUNK, 128, 4).transpose(1, 0, 2)
        # one-hot blocks: ohz[e, q, d] = ohb, ohz[d, q, 128+e] = oht
        ohcube = np.zeros((NCHUNK, 128, 128), np.float32)  # [q, e, d]
        dl2 = dlocp.reshape(NCHUNK, 128)
        valid = dl2 >= 0
        qs, es = np.nonzero(valid)
        ohcube[qs, es, dl2[qs, es]] = 1.0
        f8 = ml_dtypes.float8_e4m3fn
        ohz_np = np.empty((128, NCHUNK, 256), f8)
        ohz_np[:, :, 0:128] = ohcube.transpose(1, 0, 2).astype(f8)
        ohz_np[:, :, 128:256] = ohcube.transpose(2, 0, 1).astype(f8)
        # own dst-tile table rows for the layer-0 alpha_dst gather
        trows = np.empty((NT, 128), np.int64)
        for t in range(NT):
            base = t_of_n[i * SHARD + t * 128]
            trows[t] = base + np.arange(128)
        trw_np = _wrap_idx(trows.reshape(-1), NT * 128)
        im = {
            "xT": xT, "r0h": r0h, "r0a": r0a, "r1h": r1h, "r1a": r1a,
            "r2": r2, "r3": r3, "ident": ident, "identb": identb,
            "srcw": _wrap_idx(srcp, E_pad), "trw": trw_np,
            "ohz": ohz_np,
            "ae0": np.ascontiguousarray(ae0p),
            "ae1": np.ascontiguousarray(ae1p),
        }
        if use_b0:
            im["b0t"] = np.tile(np.asarray(b0, np.float32)[iperm][None, :], (128, 1))
        if use_b1:
            im["b1t"] = np.tile(np.asarray(b1, np.float32)[iperm][None, :], (128, 1))
        if use_l0b:
            im["l0bt"] = np.tile(np.asarray(L0b, np.float32)[None, :], (128, 1))
        if use_l1b:
            im["l1bt"] = np.tile(np.asarray(L1b, np.float32).reshape(1, 1), (128, 1))
        in_maps.append(im)

    nc = _build_program(NP, F_IN, HC, H, C, NT, K_t, FTS, GS,
                        use_b0, use_b1, use_l0b, use_l1b)
    res = run_bass_kernel_spmd(nc, in_maps, list(range(NCORES)))
    out = np.concatenate([res.results[i]["out"][:, 0] for i in range(NCORES)])
    return out[:N].astype(np.float32)
